# revision 39
# baseline (speedup 1.0000x reference)
"""Trainium2 Bass kernel for a 4-layer GCN (nn_GCNModel), SPMD across 8 NeuronCores.

Strategy (graph/data parallel per the sharding hint):
  - Nodes are partitioned across the 8 cores (6250 real rows/core, padded to
    6272 = 49 blocks of 128).  Each core owns the edges whose DESTINATION
    falls in its shard (plus that shard's self-loops), pre-sorted by
    destination block on the host and padded with zero-weight edges so every
    core sees the same per-block chunk structure (SPMD: one NEFF, 8 cores).
  - Nodes are PERMUTED on the host (round-robin over (core, dest-block)
    bins by descending in-degree) so every bin carries ~equal edge load;
    the lo/hi gather tables OVERLAP (hi base 17408) so boundary edges can
    go to either half, rounding each block's lo half to exactly full
    128-edge chunks (zero lo padding).
  - GCN normalization is computed entirely on the HOST: the full per-edge
    coefficient dis[row]*w*dis[col] is baked into precomputed one-hot M
    tiles (M[e,d] = (row_local[e]==d)*norm[e], bf16) that are STREAMED from
    DRAM per span. Building M on DVE/Act is a trap: any compute-engine
    SBUF-write traffic runs mutually exclusive with the gather DMA's SBUF
    writes and serializes the kernel; DMA-streamed M overlaps fine.
  - Per layer: each core computes h = X_shard @ W for its rows (bf16), the
    padded h table is AllGather'd to every core's DRAM, then each core
    gathers the source rows of its edges with `dma_gather` (128 rows per
    chunk; int16 indices force the lo/hi table split; separate lo/hi
    g-tiles, ~2.4k-3.2k rows per gather, greedy byte-balanced over the 4
    SWDGE queues, 8 buffers deep so all queues stay busy) and scatter-adds
    via TensorE matmul accumulation in PSUM:
    agg^T[f,d] += G[e,f]^T @ M[e,d].
  - agg^T comes out feature-major [F, nodes] — exactly the stationary
    operand layout the next layer's matmul wants; bias+ReLU ride the
    PSUM->SBUF copy on the Scalar engine (bias is per-partition there).
  - All per-core constants ship as ONE packed f32 blob (single DMA/sem) and
    each h-table store is a single DMA — TRN2 instructions carry one sync
    wait, so fan-in of DMA semaphore lanes must stay small (Bacc splits the
    rest via event semaphores).

Self-contained: hardcodes all shapes; host side only computes norms,
reorders/pads indices and concatenates shard outputs.
"""

import os
import sys

import numpy as np
import ml_dtypes

for _p in ("/opt/trn_rl_repo", "/root/.axon_site/_ro/trn_rl_repo"):
    if os.path.isdir(_p) and _p not in sys.path:
        sys.path.append(_p)

from concourse import bacc, mybir, tile  # noqa: E402
from concourse.bass_utils import run_bass_kernel_spmd  # noqa: E402

F32 = mybir.dt.float32
BF16 = mybir.dt.bfloat16
I16 = mybir.dt.int16
BF = ml_dtypes.bfloat16

N_NODES = 50000
NCORES = 8
S_REAL = N_NODES // NCORES           # 6250 real nodes per core
NBH = (S_REAL + 127) // 128          # 49 h/source blocks (128 nodes) per core
S = NBH * 128                        # 6272 padded shard size
DB = 128                             # dest-block width (M tile columns)
NBD = S // DB                        # 98 dest blocks per core
NPAD = NCORES * S                    # 50176 padded global nodes
SPLIT = 32768                        # int16 index limit for dma_gather
HI0 = NCORES * ((N_NODES // NCORES + 127) // 128 * 128) - 32768
#     = 17408: hi table base (overlaps lo: flexible
                                     # edges with src in [HI0, SPLIT) can go
                                     # to either half -> round lo to full
                                     # chunks, zero lo padding)
SPAN = 2                             # dest blocks per gather instruction
WD = 128                             # h-table row width (bf16 -> 256B rows)
MAX_IDX_PER_GATHER = 8192            # HW-validated dma_gather size
GBUFS = 8                            # gather buffers in flight
# (F_in, F_out) per layer
LAYERS = [(128, 100), (100, 100), (100, 50), (50, 6)]

TRACE = False
TRACE_KW = {}
LAST_RESULTS = None


def _wrap_idx16(flat):
    """dma_gather index layout: idx i at (partition i%16, column i//16),
    replicated to all 8 16-partition groups."""
    n = len(flat)
    assert n % 16 == 0
    w = flat.reshape(n // 16, 16).T.astype(np.int16)     # [16, n/16]
    return np.tile(w, (8, 1))                            # [128, n/16]


def _preprocess(edge_index, edge_weight):
    """Shard + sort edges by (dest block, lo/hi col half); pad each half to a
    multiple of 128 chunks with SPMD-uniform counts; group chunk columns by
    gather span: [span lo parts (by block) | span hi parts (by block)].

    Host also computes the full GCN normalization: dis = rsqrt(deg), per-edge
    normcol = dis[row]*w*dis[col] (the complete coefficient, folded into the
    one-hot M matrices on device -- no epilogue multiply needed).

    Returns (spans, block_cols, CH_TOT, idx16, rl_a, nc_a):
      spans: list of (j0, n_lo_chunks, n_hi_chunks, [blocks])
      block_cols: per block, the ordered chunk-column indices
    """
    row = edge_index[0].astype(np.int64)
    col = edge_index[1].astype(np.int64)
    w = edge_weight.astype(np.float32)
    loop = np.arange(N_NODES, dtype=np.int64)
    row = np.concatenate([row, loop])
    col = np.concatenate([col, loop])
    w = np.concatenate([w, np.ones(N_NODES, np.float32)])

    deg = np.zeros(N_NODES, np.float64)
    np.add.at(deg, row, w.astype(np.float64))
    dis_n = np.where(deg > 0, 1.0 / np.sqrt(np.maximum(deg, 1e-12)), 0.0)
    dis_n = dis_n.astype(np.float32)
    normcol = dis_n[row] * w * dis_n[col]                # full edge norm

    # Node permutation: deal nodes round-robin over the 8*NBD (core, block)
    # bins in descending in-degree order, so every bin carries ~equal edge
    # load (kills the max-over-cores chunk padding).
    edeg = np.ones(N_NODES, np.int64)
    np.add.at(edeg, edge_index[0].astype(np.int64), 1)
    rank = np.argsort(-edeg, kind="stable")
    NBINS = NCORES * NBD
    pos = np.empty(N_NODES, np.int64)        # node -> global padded position
    bin_of = np.arange(N_NODES) % NBINS
    slot_of = np.arange(N_NODES) // NBINS
    cores_of = bin_of % NCORES
    blk_of = bin_of // NCORES
    pos[rank] = cores_of * S + blk_of * DB + slot_of
    assert slot_of.max() < DB
    ids = np.full((NCORES, S), -1, np.int64)  # (core, local pos) -> node
    ids[pos // S, pos % S] = np.arange(N_NODES)

    posr = pos[row]
    core = posr // S
    r_in = posr % S
    blk = r_in // DB
    rloc = (r_in % DB).astype(np.float32)
    colp = pos[col]

    # Flexible lo/hi assignment: within each (core, block), rank edges by
    # source position; the first 128*K_b go lo (K_b = max over cores of
    # ceil(#forced_lo/128), forced_lo = src < HI0), the rest hi.  All lo
    # chunks are exactly full; only hi carries padding.
    grp = core * NBD + blk
    order0 = np.lexsort((colp, grp))
    gsort = grp[order0]
    starts = np.searchsorted(gsort, np.arange(NCORES * NBD))
    rank = np.empty(len(grp), np.int64)
    rank[order0] = np.arange(len(grp)) - starts[gsort]

    counts_flo = np.zeros((NCORES, NBD), np.int64)
    np.add.at(counts_flo, (core[colp < HI0], blk[colp < HI0]), 1)
    K_b = -(-counts_flo.max(axis=0) // 128)              # [NBD] lo chunks
    counts_tot = np.zeros((NCORES, NBD), np.int64)
    np.add.at(counts_tot, (core, blk), 1)
    assert (counts_tot >= 128 * K_b[None, :]).all()

    hi = (rank >= 128 * K_b[blk]).astype(np.int64)
    # hi-assigned edges must lie in the hi table's range
    assert (colp[hi == 1] >= HI0).all()
    assert (colp[hi == 0] < SPLIT).all()

    CHP = np.stack([K_b, -(-np.maximum(
        counts_tot - 128 * K_b[None, :], 0).max(axis=0) // 128)], axis=1)
    CH_TOT = int(CHP.sum())

    # span/chunk-column layout
    spans = []
    block_cols = [[] for _ in range(NBD)]
    part_col = {}                                        # (b, h) -> start col
    j = 0
    for s0 in range(0, NBD, SPAN):
        blocks = list(range(s0, min(s0 + SPAN, NBD)))
        j0 = j
        n_lo = n_hi = 0
        for h in range(2):
            for b in blocks:
                nch = int(CHP[b, h])
                part_col[(b, h)] = j
                block_cols[b].extend(range(j, j + nch))
                j += nch
                if h == 0:
                    n_lo += nch
                else:
                    n_hi += nch
        assert n_lo * 128 <= MAX_IDX_PER_GATHER
        assert n_hi * 128 <= MAX_IDX_PER_GATHER
        spans.append((j0, n_lo, n_hi, blocks))
    assert j == CH_TOT

    idx16 = np.zeros((NCORES, 128, CH_TOT * 8), np.int16)
    rl_a = np.zeros((NCORES, 128, CH_TOT), np.float32)
    nc_a = np.zeros((NCORES, 128, CH_TOT), np.float32)
    mtab = np.zeros((NCORES, 128, CH_TOT * DB), BF)

    order = np.lexsort((hi, blk, core))
    scolp, srloc, snc = colp[order], rloc[order], normcol[order]
    score, sblk, shi = core[order], blk[order], hi[order]
    bounds = np.searchsorted(
        score * (NBD * 2) + sblk * 2 + shi, np.arange(NCORES * NBD * 2 + 1))

    for c in range(NCORES):
        for b in range(NBD):
            for h in range(2):
                k = (c * NBD + b) * 2 + h
                s0, s1 = bounds[k], bounds[k + 1]
                nch = int(CHP[b, h])
                if nch == 0:
                    assert s1 == s0
                    continue
                n = nch * 128
                jc = part_col[(b, h)]
                lim = SPLIT if h == 0 else NPAD - HI0
                # spread padding indices so they don't hammer row 0
                ii = (np.arange(n, dtype=np.int64) * 97 + jc * 131) % lim
                ii[:s1 - s0] = scolp[s0:s1] - h * HI0
                rr = np.zeros(n, np.float32)
                rr[:s1 - s0] = srloc[s0:s1]
                nn = np.zeros(n, np.float32)
                nn[:s1 - s0] = snc[s0:s1]
                rl_a[c, :, jc:jc + nch] = rr.reshape(nch, 128).T
                nc_a[c, :, jc:jc + nch] = nn.reshape(nch, 128).T
                idx16[c, :, jc * 8:jc * 8 + n // 16] = _wrap_idx16(ii)
    # M tiles (layer-independent): mtab[:, j*DB:(j+1)*DB] = one-hot(rl)*norm
    dDB = np.arange(DB, dtype=np.float32)[None, None, :]
    for c in range(NCORES):
        m = (rl_a[c][:, :, None] == dDB) * nc_a[c][:, :, None]
        mtab[c] = m.astype(BF).reshape(128, CH_TOT * DB)
    return dict(spans=spans, block_cols=block_cols, CH_TOT=CH_TOT,
                idx16=idx16, rl_a=rl_a, nc_a=nc_a, mtab=mtab, ids=ids)


def _blob_offsets(CH_TOT):
    """Column layout of the packed per-core constant blob [128, NCOLS] f32."""
    off = {}
    o = 0
    for k, n in (("idx16", CH_TOT * 4), ("rl", CH_TOT), ("norm", CH_TOT),
                 ("iota", 64), ("xT", S // 2)):
        off[k] = o
        o += n
    for l, (fin, fout) in enumerate(LAYERS):
        off[f"W{l}"] = o
        o += fout // 2
    for l, (fin, fout) in enumerate(LAYERS):
        off[f"b{l}"] = o
        o += 1
    return off, o


def _build(pre, mode="full"):
    spans = pre["spans"]
    block_cols = pre["block_cols"]
    CH_TOT = pre["CH_TOT"]
    # mode: "<base>[-repN]" where base is one of
    #   full  - real kernel
    #   nodeg - alias of full (deg phase is host-side now)
    #   nocc  - collectives -> local DMA (wrong numerics; timing ablation)
    #   noagg - skip gather+aggregation entirely (ablation)
    #   nogat - skip the dma_gather instructions only (ablation)
    #   min   - minimal output path
    # repN repeats the whole body N times for marginal-time measurement.
    reps = 1
    if "-rep" in mode:
        mode, _, r = mode.partition("-rep")
        reps = int(r)
    elif mode.startswith("rep"):
        reps, mode = int(mode[3:]), "full"
    if mode == "nodeg":
        mode = "full"
    nc = bacc.Bacc(None, num_devices=NCORES, num_swdge_queues=4)
    AGRP = [list(range(NCORES))]
    AF = mybir.ActivationFunctionType
    OP = mybir.AluOpType

    off, ncols = _blob_offsets(CH_TOT)
    blob_p = nc.declare_dram_parameter("blob", [128, ncols], F32, isOutput=False)
    out_p = nc.declare_dram_parameter("out", [LAYERS[-1][1], S], F32, isOutput=True)
    tabx_p = (nc.declare_dram_parameter("tabx", [SPLIT, WD], BF16,
                                        isOutput=False)
              if mode.startswith("gat") and mode != "gatonly" else None)
    mtab_p = nc.declare_dram_parameter("mtab", [128, CH_TOT * DB // 2], F32,
                                       isOutput=False)

    with tile.TileContext(nc, num_cores=NCORES) as tc:
        with (
            tc.tile_pool(name="const", bufs=1) as cpool,
            tc.tile_pool(name="xpool", bufs=2) as xpool,
            tc.tile_pool(name="dram", bufs=1, space="DRAM") as dpool,
            tc.tile_pool(name="psum", bufs=2, space="PSUM") as ppool,
            tc.tile_pool(name="work", bufs=3) as wpool,
        ):
            blob = cpool.tile([128, ncols], F32)
            nc.sync.dma_start(out=blob[:], in_=blob_p[:])

            idx16_sb = blob[:, off["idx16"]:off["idx16"] + CH_TOT * 4].bitcast(I16)
            rl_sb = blob[:, off["rl"]:off["rl"] + CH_TOT]
            norm_sb = blob[:, off["norm"]:off["norm"] + CH_TOT]
            iota_b = blob[:, off["iota"]:off["iota"] + 64].bitcast(BF16)
            xT0 = blob[:, off["xT"]:off["xT"] + S // 2].bitcast(BF16)
            W_sb = [blob[:LAYERS[l][0],
                         off[f"W{l}"]:off[f"W{l}"] + LAYERS[l][1] // 2].bitcast(BF16)
                    for l in range(4)]
            b_sb = [blob[:LAYERS[l][1], off[f"b{l}"]:off[f"b{l}"] + 1]
                    for l in range(4)]

            if mode == "min":
                o_min = xpool.tile([LAYERS[-1][1], S], F32, tag="xT")
                nc.vector.memset(o_min[:], 0.25)
                for _rep in range(reps):
                    nc.vector.tensor_tensor(
                        o_min[0:1, 0:1], blob[0:1, 0:1], blob[0:1, 0:1],
                        OP.mult)
                nc.sync.dma_start(out=out_p[:], in_=o_min[:])

            gq = [0]                 # strict round-robin SWDGE queue counter
            qload = [0, 0, 0, 0]     # greedy byte-balancing across queues

            def pick_q(nch):
                q = qload.index(min(qload))
                qload[q] += nch
                return q
            gconst = None
            if mode == "gatcomp":
                gconst = cpool.tile([128, WD], BF16)
                nc.vector.memset(gconst[:], 0.01)

            # body repetition for precise marginal-time measurement (repN)
            for _rep in range(reps if mode != "min" else 0):
                if mode in ("gatbare", "gatdve", "gatdve2", "gatact", "gatpe"):
                    o_gb = xpool.tile([LAYERS[-1][1], S], F32, tag="xT")
                    nc.vector.memset(o_gb[:], 0.25)
                    gc2 = cpool.tile([128, WD], BF16, name="gc2")
                    nc.vector.memset(gc2[:], 0.01)
                    mc2 = cpool.tile([128, 128], BF16, name="mc2")
                    nc.vector.memset(mc2[:], 0.01)
                    for l in range(4):
                        fout = LAYERS[l][1]
                        for si, (j0, n_lo, n_hi, blocks) in enumerate(spans):
                            glo = wpool.tile([128, max(n_lo, 1) * WD], BF16,
                                             tag="glo", bufs=GBUFS)
                            ghi = wpool.tile([128, max(n_hi, 1) * WD], BF16,
                                             tag="ghi", bufs=GBUFS)
                            for h, gt, nch in ((0, glo, n_lo), (1, ghi, n_hi)):
                                if nch == 0:
                                    continue
                                i16 = (j0 + (0 if h == 0 else n_lo)) * 8
                                nc.gpsimd.dma_gather(
                                    gt[:, 0:nch * WD].rearrange(
                                        "p (c w) -> p c w", w=WD),
                                    tabx_p[:], idx16_sb[:, i16:i16 + nch * 8],
                                    nch * 128, nch * 128, WD,
                                    single_packet=False, queue_num=gq[0] % 4)
                                gq[0] += 1
                            if mode == "gatdve":
                                for b in blocks:
                                    for k, j in enumerate(block_cols[b]):
                                        m = wpool.tile([128, 128], BF16,
                                                       tag="m", bufs=16)
                                        nc.vector.tensor_scalar(
                                            m[:], iota_b, rl_sb[:, j:j + 1],
                                            norm_sb[:, j:j + 1],
                                            OP.is_equal, OP.mult)
                            elif mode == "gatdve2":
                                for b in blocks:
                                    for k, j in enumerate(block_cols[b]):
                                        m = ppool.tile([128, 128], BF16,
                                                       tag="mps", bufs=8)
                                        nc.vector.tensor_scalar(
                                            m[:], iota_b, rl_sb[:, j:j + 1],
                                            norm_sb[:, j:j + 1],
                                            OP.is_equal, OP.mult)
                            elif mode == "gatact":
                                for b in blocks:
                                    for k, j in enumerate(block_cols[b]):
                                        m = wpool.tile([128, 128], BF16,
                                                       tag="m", bufs=16)
                                        nc.scalar.activation(
                                            m[:], mc2[:],
                                            mybir.ActivationFunctionType.Copy)
                            elif mode == "gatpe":
                                for b in blocks:
                                    cols = block_cols[b]
                                    ps_a = ppool.tile([fout, 128], F32,
                                                      tag="psa", bufs=4)
                                    for k, j in enumerate(cols):
                                        nc.tensor.matmul(
                                            out=ps_a[:],
                                            lhsT=gc2[:, :fout], rhs=mc2[:],
                                            start=(k == 0),
                                            stop=(k == len(cols) - 1))
                    nc.vector.tensor_tensor(
                        o_gb[0:1, 0:1], ghi[0:1, 0:1], ghi[0:1, 0:1], OP.mult)
                    nc.sync.dma_start(out=out_p[:], in_=o_gb[:])
                    continue
                xT = xT0
                # prologue: h for layer 0 from the input xT0
                h_all = wpool.tile([128, NBH * WD], BF16, tag="hall", bufs=2)
                for b in range(NBH):
                    ps_h = ppool.tile([128, LAYERS[0][1]], F32, tag="ps_small")
                    nc.tensor.matmul(
                        out=ps_h[:], lhsT=xT0[:LAYERS[0][0],
                                             b * 128:(b + 1) * 128],
                        rhs=W_sb[0], start=True, stop=True)
                    nc.scalar.activation(
                        h_all[:, b * WD:b * WD + LAYERS[0][1]], ps_h[:],
                        AF.Copy)
                for l, (fin, fout) in enumerate(LAYERS):
                    hsh = dpool.tile([S, WD], BF16, name=f"hsh{l}_{_rep}")
                    nc.sync.dma_start(
                        out=hsh.rearrange("(b p) w -> p b w", p=128),
                        in_=h_all[:].rearrange("p (b w) -> p b w", w=WD))

                    htab = dpool.tile([NPAD, WD], BF16, addr_space="Shared",
                                      name=f"htab{l}_{_rep}")
                    if mode == "nocc":
                        nc.sync.dma_start(out=htab[0:S, :], in_=hsh[:])
                    else:
                        nc.gpsimd.collective_compute(
                            "AllGather", OP.bypass, replica_groups=AGRP,
                            ins=[hsh[:]], outs=[htab[:]])

                    # aggregation: agg^T[f, d] += G[e, f]^T @ onehot[e, d]
                    xT_next = xpool.tile([fout, S], BF16 if l + 1 < len(LAYERS)
                                         else F32, tag="xT", name=f"xT{l + 1}_sb")
                    h_next = (wpool.tile([128, NBH * WD], BF16, tag="hall",
                                         bufs=2, name=f"hnext{l}_{_rep}")
                              if l + 1 < len(LAYERS) else None)
                    if mode == "noagg":
                        nc.vector.memset(xT_next[:], 0.25)
                        xT = xT_next
                        continue
                    if mode in ("gatonly", "gatpar"):
                        nc.vector.memset(xT_next[:], 0.25)
                        for si, (j0, n_lo, n_hi, blocks) in enumerate(spans):
                            nch_s = n_lo + n_hi
                            g = wpool.tile([128, nch_s * WD], BF16, tag="g",
                                           bufs=GBUFS)
                            tlo = htab[0:SPLIT, :] if mode == "gatonly" \
                                else tabx_p[:]
                            thi = htab[SPLIT:NPAD, :] if mode == "gatonly" \
                                else tabx_p[:]
                            for h, tab, o0, nch in (
                                    (0, tlo, 0, n_lo),
                                    (1, thi, n_lo * WD, n_hi)):
                                if nch == 0:
                                    continue
                                i16 = (j0 + (0 if h == 0 else n_lo)) * 8
                                nc.gpsimd.dma_gather(
                                    g[:, o0:o0 + nch * WD].rearrange(
                                        "p (c w) -> p c w", w=WD),
                                    tab, idx16_sb[:, i16:i16 + nch * 8],
                                    nch * 128, nch * 128, WD,
                                    single_packet=False, queue_num=gq[0] % 4)
                                gq[0] += 1
                        xT = xT_next
                        continue
                    for si, (j0, n_lo, n_hi, blocks) in enumerate(spans):
                        glo = wpool.tile([128, max(n_lo, 1) * WD], BF16,
                                         tag="glo", bufs=GBUFS)
                        ghi = wpool.tile([128, max(n_hi, 1) * WD], BF16,
                                         tag="ghi", bufs=GBUFS)
                        for h, tab, gt, nch in ((0, htab[0:SPLIT, :], glo, n_lo),
                                                (1, htab[HI0:NPAD, :],
                                                 ghi, n_hi)):
                            if nch == 0:
                                continue
                            if mode == "nogat":
                                nc.vector.memset(gt[:, 0:1], 0.25)
                                continue
                            if mode == "gatcomp":
                                tab = tabx_p[:]
                            i16 = (j0 + (0 if h == 0 else n_lo)) * 8
                            nc.gpsimd.dma_gather(
                                gt[:, 0:nch * WD].rearrange(
                                    "p (c w) -> p c w", w=WD),
                                tab, idx16_sb[:, i16:i16 + nch * 8],
                                nch * 128, nch * 128, WD,
                                single_packet=False,
                                queue_num=0 if os.environ.get("GQ1")
                                else pick_q(nch))
                            gq[0] += 1
                        nch_s = n_lo + n_hi
                        m_span = wpool.tile([128, nch_s * DB], BF16,
                                            tag="m", bufs=3)
                        nc.sync.dma_start(
                            out=m_span[:],
                            in_=mtab_p[:, j0 * DB // 2:(j0 + nch_s) * DB // 2]
                            .bitcast(BF16))
                        for b in blocks:
                            cols = block_cols[b]
                            ps_a = ppool.tile([fout, DB], F32, tag="psa", bufs=4)
                            for k, j in enumerate(cols):
                                if mode == "gatcomp":
                                    gsrc, o0 = gconst, 0
                                elif j - j0 < n_lo:
                                    gsrc, o0 = glo, (j - j0) * WD
                                else:
                                    gsrc, o0 = ghi, (j - j0 - n_lo) * WD
                                nc.tensor.matmul(
                                    out=ps_a[:], lhsT=gsrc[:, o0:o0 + fout],
                                    rhs=m_span[:, (j - j0) * DB:
                                               (j - j0 + 1) * DB],
                                    start=(k == 0),
                                    stop=(k == len(cols) - 1))
                            # epilogue: +bias, ReLU (norm fully in M)
                            nc.scalar.activation(
                                xT_next[:, b * DB:(b + 1) * DB], ps_a[:],
                                AF.Relu if l + 1 < len(LAYERS) else AF.Identity,
                                bias=b_sb[l][:, 0:1])
                            if l + 1 < len(LAYERS):
                                # next layer's h for this block rides the agg
                                # phase (PE/Act have slack; hides the h phase
                                # and starts the collective earlier)
                                fin2, fout2 = LAYERS[l + 1]
                                ps_h = ppool.tile([128, fout2], F32,
                                                  tag="ps_small")
                                nc.tensor.matmul(
                                    out=ps_h[:],
                                    lhsT=xT_next[:fin2,
                                                 b * 128:(b + 1) * 128],
                                    rhs=W_sb[l + 1], start=True, stop=True)
                                nc.scalar.activation(
                                    h_next[:, b * WD:b * WD + fout2],
                                    ps_h[:], AF.Copy)
                    xT = xT_next
                    h_all = h_next

                nc.sync.dma_start(out=out_p[:], in_=xT[:])
    nc.finalize()   # Bacc: reg alloc + event-sem wait splitting
    return nc


def _make_in_maps(x, pre, W0, b0, W1, b1, W2, b2, W3, b3):
    CH_TOT, idx16, mtab, ids = (pre["CH_TOT"], pre["idx16"], pre["mtab"],
                                pre["ids"])
    rl_a, nc_a = pre["rl_a"], pre["nc_a"]
    off, ncols = _blob_offsets(CH_TOT)
    Ws = [np.asarray(W, np.float32).astype(BF) for W in (W0, W1, W2, W3)]
    bs = [np.asarray(b, np.float32).reshape(-1) for b in (b0, b1, b2, b3)]

    in_maps = []
    for c in range(NCORES):
        blob = np.zeros((128, ncols), np.float32)
        blob[:, off["idx16"]:off["idx16"] + CH_TOT * 4] = idx16[c].view(np.float32)
        blob[:, off["rl"]:off["rl"] + CH_TOT] = rl_a[c]
        blob[:, off["norm"]:off["norm"] + CH_TOT] = nc_a[c]
        blob[:, off["iota"]:off["iota"] + 64] = np.broadcast_to(
            np.arange(128, dtype=np.float32).astype(BF).view(np.float32)[None, :],
            (128, 64))
        xb = np.zeros((128, S), BF)
        valid = ids[c] >= 0
        xb[:, valid] = x[ids[c][valid]].T.astype(BF)
        blob[:, off["xT"]:off["xT"] + S // 2] = xb.view(np.float32)
        for l, (fin, fout) in enumerate(LAYERS):
            blob[:fin, off[f"W{l}"]:off[f"W{l}"] + fout // 2] = Ws[l].view(np.float32)
            blob[:fout, off[f"b{l}"]] = bs[l]
        in_maps.append({"blob": blob,
                        "mtab": mtab[c].view(np.float32)})
    return in_maps


def kernel(x, edge_index, edge_weight, W0, b0, W1, b1, W2, b2, W3, b3):
    global LAST_RESULTS
    x = np.ascontiguousarray(np.asarray(x, np.float32))
    pre = _preprocess(np.asarray(edge_index), np.asarray(edge_weight))

    nc = _build(pre)

    in_maps = _make_in_maps(x, pre, W0, b0, W1, b1, W2, b2, W3, b3)

    res = run_bass_kernel_spmd(nc, in_maps, core_ids=list(range(NCORES)),
                               trace=TRACE, trace_kwargs=dict(TRACE_KW))
    LAST_RESULTS = res
    ids = pre["ids"]
    out = np.zeros((N_NODES, LAYERS[-1][1]), np.float32)
    for c in range(NCORES):
        valid = ids[c] >= 0
        out[ids[c][valid]] = res.results[c]["out"][:, valid].T
    return np.ascontiguousarray(out)


# revision 40
# speedup vs baseline: 1.0827x; 1.0827x over previous
"""Trainium2 Bass kernel for a 4-layer GCN (nn_GCNModel), SPMD across 8 NeuronCores.

Strategy (graph/data parallel per the sharding hint):
  - Nodes are partitioned across the 8 cores (6250 real rows/core, padded to
    6272 = 49 blocks of 128).  Each core owns the edges whose DESTINATION
    falls in its shard (plus that shard's self-loops), pre-sorted by
    destination block on the host and padded with zero-weight edges so every
    core sees the same per-block chunk structure (SPMD: one NEFF, 8 cores).
  - Nodes are PERMUTED on the host (round-robin over (core, dest-block)
    bins by descending in-degree) so every bin carries ~equal edge load;
    the lo/hi gather tables OVERLAP (hi base 17408) so boundary edges can
    go to either half, rounding each block's lo half to exactly full
    128-edge chunks (zero lo padding).
  - GCN normalization is computed entirely on the HOST: the full per-edge
    coefficient dis[row]*w*dis[col] is baked into precomputed one-hot M
    tiles (M[e,d] = (row_local[e]==d)*norm[e], bf16) that are STREAMED from
    DRAM per span. Building M on DVE/Act is a trap: any compute-engine
    SBUF-write traffic runs mutually exclusive with the gather DMA's SBUF
    writes and serializes the kernel; DMA-streamed M overlaps fine.
  - Per layer: each core computes h = X_shard @ W for its rows (bf16), the
    padded h table is AllGather'd to every core's DRAM, then each core
    gathers the source rows of its edges with `dma_gather` (128 rows per
    chunk; int16 indices force the lo/hi table split; separate lo/hi
    g-tiles, ~2.4k-3.2k rows per gather, greedy byte-balanced over the 4
    SWDGE queues, 8 buffers deep so all queues stay busy) and scatter-adds
    via TensorE matmul accumulation in PSUM:
    agg^T[f,d] += G[e,f]^T @ M[e,d].
  - agg^T comes out feature-major [F, nodes] — exactly the stationary
    operand layout the next layer's matmul wants; bias+ReLU ride the
    PSUM->SBUF copy on the Scalar engine (bias is per-partition there).
  - All per-core constants ship as ONE packed f32 blob (single DMA/sem) and
    each h-table store is a single DMA — TRN2 instructions carry one sync
    wait, so fan-in of DMA semaphore lanes must stay small (Bacc splits the
    rest via event semaphores).

Self-contained: hardcodes all shapes; host side only computes norms,
reorders/pads indices and concatenates shard outputs.

Known remaining headroom (untried/unfinished):
  - M tiles are mostly zeros: sorting each (block, half)'s edges by dest and
    cutting chunks at fixed 16-dest (lo) / 8-dest (hi) ranges would let M
    stream ~8x fewer bytes (psum sub-range writes with start=True on the
    covering lo chunk, start=False accumulation for hi), trading ~+30%
    gather padding -- est. -250us at the ~200 GB/s SBUF DMA-write wall.
  - The per-layer AllGather (~40us/layer) is serial; chunked collectives
    could hide most of it.
"""

import os
import sys

import numpy as np
import ml_dtypes

for _p in ("/opt/trn_rl_repo", "/root/.axon_site/_ro/trn_rl_repo"):
    if os.path.isdir(_p) and _p not in sys.path:
        sys.path.append(_p)

from concourse import bacc, mybir, tile  # noqa: E402
from concourse.bass_utils import run_bass_kernel_spmd  # noqa: E402

F32 = mybir.dt.float32
BF16 = mybir.dt.bfloat16
I16 = mybir.dt.int16
BF = ml_dtypes.bfloat16

N_NODES = 50000
NCORES = 8
S_REAL = N_NODES // NCORES           # 6250 real nodes per core
NBH = (S_REAL + 127) // 128          # 49 h/source blocks (128 nodes) per core
S = NBH * 128                        # 6272 padded shard size
DB = 128                             # dest-block width (M tile columns)
NBD = S // DB                        # 98 dest blocks per core
NPAD = NCORES * S                    # 50176 padded global nodes
SPLIT = 32768                        # int16 index limit for dma_gather
HI0 = NCORES * ((N_NODES // NCORES + 127) // 128 * 128) - 32768
#     = 17408: hi table base (overlaps lo: flexible
                                     # edges with src in [HI0, SPLIT) can go
                                     # to either half -> round lo to full
                                     # chunks, zero lo padding)
SPAN = 2                             # dest blocks per gather instruction
WD = 128                             # h-table row width (bf16 -> 256B rows)
MAX_IDX_PER_GATHER = 8192            # HW-validated dma_gather size
GBUFS = 8                            # gather buffers in flight
# (F_in, F_out) per layer
LAYERS = [(128, 100), (100, 100), (100, 50), (50, 6)]

TRACE = False
TRACE_KW = {}
LAST_RESULTS = None


def _wrap_idx16(flat):
    """dma_gather index layout: idx i at (partition i%16, column i//16),
    replicated to all 8 16-partition groups."""
    n = len(flat)
    assert n % 16 == 0
    w = flat.reshape(n // 16, 16).T.astype(np.int16)     # [16, n/16]
    return np.tile(w, (8, 1))                            # [128, n/16]


def _preprocess(edge_index, edge_weight):
    """Shard + sort edges by (dest block, lo/hi col half); pad each half to a
    multiple of 128 chunks with SPMD-uniform counts; group chunk columns by
    gather span: [span lo parts (by block) | span hi parts (by block)].

    Host also computes the full GCN normalization: dis = rsqrt(deg), per-edge
    normcol = dis[row]*w*dis[col] (the complete coefficient, folded into the
    one-hot M matrices on device -- no epilogue multiply needed).

    Returns (spans, block_cols, CH_TOT, idx16, rl_a, nc_a):
      spans: list of (j0, n_lo_chunks, n_hi_chunks, [blocks])
      block_cols: per block, the ordered chunk-column indices
    """
    row = edge_index[0].astype(np.int64)
    col = edge_index[1].astype(np.int64)
    w = edge_weight.astype(np.float32)
    loop = np.arange(N_NODES, dtype=np.int64)
    row = np.concatenate([row, loop])
    col = np.concatenate([col, loop])
    w = np.concatenate([w, np.ones(N_NODES, np.float32)])

    deg = np.zeros(N_NODES, np.float64)
    np.add.at(deg, row, w.astype(np.float64))
    dis_n = np.where(deg > 0, 1.0 / np.sqrt(np.maximum(deg, 1e-12)), 0.0)
    dis_n = dis_n.astype(np.float32)
    normcol = dis_n[row] * w * dis_n[col]                # full edge norm

    # Node permutation: deal nodes round-robin over the 8*NBD (core, block)
    # bins in descending in-degree order, so every bin carries ~equal edge
    # load (kills the max-over-cores chunk padding).
    edeg = np.ones(N_NODES, np.int64)
    np.add.at(edeg, edge_index[0].astype(np.int64), 1)
    rank = np.argsort(-edeg, kind="stable")
    NBINS = NCORES * NBD
    pos = np.empty(N_NODES, np.int64)        # node -> global padded position
    bin_of = np.arange(N_NODES) % NBINS
    slot_of = np.arange(N_NODES) // NBINS
    cores_of = bin_of % NCORES
    blk_of = bin_of // NCORES
    pos[rank] = cores_of * S + blk_of * DB + slot_of
    assert slot_of.max() < DB
    ids = np.full((NCORES, S), -1, np.int64)  # (core, local pos) -> node
    ids[pos // S, pos % S] = np.arange(N_NODES)

    posr = pos[row]
    core = posr // S
    r_in = posr % S
    blk = r_in // DB
    rloc = (r_in % DB).astype(np.float32)
    colp = pos[col]

    # Flexible lo/hi assignment: within each (core, block), rank edges by
    # source position; the first 128*K_b go lo (K_b = max over cores of
    # ceil(#forced_lo/128), forced_lo = src < HI0), the rest hi.  All lo
    # chunks are exactly full; only hi carries padding.
    grp = core * NBD + blk
    order0 = np.lexsort((colp, grp))
    gsort = grp[order0]
    starts = np.searchsorted(gsort, np.arange(NCORES * NBD))
    rank = np.empty(len(grp), np.int64)
    rank[order0] = np.arange(len(grp)) - starts[gsort]

    counts_flo = np.zeros((NCORES, NBD), np.int64)
    np.add.at(counts_flo, (core[colp < HI0], blk[colp < HI0]), 1)
    K_b = -(-counts_flo.max(axis=0) // 128)              # [NBD] lo chunks
    counts_tot = np.zeros((NCORES, NBD), np.int64)
    np.add.at(counts_tot, (core, blk), 1)
    assert (counts_tot >= 128 * K_b[None, :]).all()

    hi = (rank >= 128 * K_b[blk]).astype(np.int64)
    # hi-assigned edges must lie in the hi table's range
    assert (colp[hi == 1] >= HI0).all()
    assert (colp[hi == 0] < SPLIT).all()

    CHP = np.stack([K_b, -(-np.maximum(
        counts_tot - 128 * K_b[None, :], 0).max(axis=0) // 128)], axis=1)
    CH_TOT = int(CHP.sum())

    # span/chunk-column layout
    spans = []
    block_cols = [[] for _ in range(NBD)]
    part_col = {}                                        # (b, h) -> start col
    j = 0
    for s0 in range(0, NBD, SPAN):
        blocks = list(range(s0, min(s0 + SPAN, NBD)))
        j0 = j
        n_lo = n_hi = 0
        for h in range(2):
            for b in blocks:
                nch = int(CHP[b, h])
                part_col[(b, h)] = j
                block_cols[b].extend(range(j, j + nch))
                j += nch
                if h == 0:
                    n_lo += nch
                else:
                    n_hi += nch
        assert n_lo * 128 <= MAX_IDX_PER_GATHER
        assert n_hi * 128 <= MAX_IDX_PER_GATHER
        spans.append((j0, n_lo, n_hi, blocks))
    assert j == CH_TOT

    idx16 = np.zeros((NCORES, 128, CH_TOT * 8), np.int16)
    rl_a = np.zeros((NCORES, 128, CH_TOT), np.float32)
    nc_a = np.zeros((NCORES, 128, CH_TOT), np.float32)
    mtab = np.zeros((NCORES, 128, CH_TOT * DB), BF)

    order = np.lexsort((hi, blk, core))
    scolp, srloc, snc = colp[order], rloc[order], normcol[order]
    score, sblk, shi = core[order], blk[order], hi[order]
    bounds = np.searchsorted(
        score * (NBD * 2) + sblk * 2 + shi, np.arange(NCORES * NBD * 2 + 1))

    for c in range(NCORES):
        for b in range(NBD):
            for h in range(2):
                k = (c * NBD + b) * 2 + h
                s0, s1 = bounds[k], bounds[k + 1]
                nch = int(CHP[b, h])
                if nch == 0:
                    assert s1 == s0
                    continue
                n = nch * 128
                jc = part_col[(b, h)]
                lim = SPLIT if h == 0 else NPAD - HI0
                # spread padding indices so they don't hammer row 0
                ii = (np.arange(n, dtype=np.int64) * 97 + jc * 131) % lim
                ii[:s1 - s0] = scolp[s0:s1] - h * HI0
                rr = np.zeros(n, np.float32)
                rr[:s1 - s0] = srloc[s0:s1]
                nn = np.zeros(n, np.float32)
                nn[:s1 - s0] = snc[s0:s1]
                rl_a[c, :, jc:jc + nch] = rr.reshape(nch, 128).T
                nc_a[c, :, jc:jc + nch] = nn.reshape(nch, 128).T
                idx16[c, :, jc * 8:jc * 8 + n // 16] = _wrap_idx16(ii)
    # M tiles (layer-independent): mtab[:, j*DB:(j+1)*DB] = one-hot(rl)*norm
    dDB = np.arange(DB, dtype=np.float32)[None, None, :]
    for c in range(NCORES):
        m = (rl_a[c][:, :, None] == dDB) * nc_a[c][:, :, None]
        mtab[c] = m.astype(BF).reshape(128, CH_TOT * DB)
    return dict(spans=spans, block_cols=block_cols, CH_TOT=CH_TOT,
                idx16=idx16, rl_a=rl_a, nc_a=nc_a, mtab=mtab, ids=ids)


def _blob_offsets(CH_TOT):
    """Column layout of the packed per-core constant blob [128, NCOLS] f32."""
    off = {}
    o = 0
    for k, n in (("idx16", CH_TOT * 4), ("rl", CH_TOT), ("norm", CH_TOT),
                 ("iota", 64), ("xT", S // 2)):
        off[k] = o
        o += n
    for l, (fin, fout) in enumerate(LAYERS):
        off[f"W{l}"] = o
        o += fout // 2
    for l, (fin, fout) in enumerate(LAYERS):
        off[f"b{l}"] = o
        o += 1
    return off, o


def _build(pre, mode="full"):
    spans = pre["spans"]
    block_cols = pre["block_cols"]
    CH_TOT = pre["CH_TOT"]
    # mode: "<base>[-repN]" where base is one of
    #   full  - real kernel
    #   nodeg - alias of full (deg phase is host-side now)
    #   nocc  - collectives -> local DMA (wrong numerics; timing ablation)
    #   noagg - skip gather+aggregation entirely (ablation)
    #   nogat - skip the dma_gather instructions only (ablation)
    #   min   - minimal output path
    # repN repeats the whole body N times for marginal-time measurement.
    reps = 1
    if "-rep" in mode:
        mode, _, r = mode.partition("-rep")
        reps = int(r)
    elif mode.startswith("rep"):
        reps, mode = int(mode[3:]), "full"
    if mode == "nodeg":
        mode = "full"
    nc = bacc.Bacc(None, num_devices=NCORES, num_swdge_queues=4)
    AGRP = [list(range(NCORES))]
    AF = mybir.ActivationFunctionType
    OP = mybir.AluOpType

    off, ncols = _blob_offsets(CH_TOT)
    blob_p = nc.declare_dram_parameter("blob", [128, ncols], F32, isOutput=False)
    out_p = nc.declare_dram_parameter("out", [LAYERS[-1][1], S], F32, isOutput=True)
    tabx_p = (nc.declare_dram_parameter("tabx", [SPLIT, WD], BF16,
                                        isOutput=False)
              if mode.startswith("gat") and mode != "gatonly" else None)
    mtab_p = nc.declare_dram_parameter("mtab", [128, CH_TOT * DB // 2], F32,
                                       isOutput=False)

    with tile.TileContext(nc, num_cores=NCORES) as tc:
        with (
            tc.tile_pool(name="const", bufs=1) as cpool,
            tc.tile_pool(name="xpool", bufs=2) as xpool,
            tc.tile_pool(name="dram", bufs=1, space="DRAM") as dpool,
            tc.tile_pool(name="psum", bufs=2, space="PSUM") as ppool,
            tc.tile_pool(name="work", bufs=3) as wpool,
        ):
            blob = cpool.tile([128, ncols], F32)
            nc.sync.dma_start(out=blob[:], in_=blob_p[:])

            idx16_sb = blob[:, off["idx16"]:off["idx16"] + CH_TOT * 4].bitcast(I16)
            rl_sb = blob[:, off["rl"]:off["rl"] + CH_TOT]
            norm_sb = blob[:, off["norm"]:off["norm"] + CH_TOT]
            iota_b = blob[:, off["iota"]:off["iota"] + 64].bitcast(BF16)
            xT0 = blob[:, off["xT"]:off["xT"] + S // 2].bitcast(BF16)
            W_sb = [blob[:LAYERS[l][0],
                         off[f"W{l}"]:off[f"W{l}"] + LAYERS[l][1] // 2].bitcast(BF16)
                    for l in range(4)]
            b_sb = [blob[:LAYERS[l][1], off[f"b{l}"]:off[f"b{l}"] + 1]
                    for l in range(4)]

            if mode == "min":
                o_min = xpool.tile([LAYERS[-1][1], S], F32, tag="xT")
                nc.vector.memset(o_min[:], 0.25)
                for _rep in range(reps):
                    nc.vector.tensor_tensor(
                        o_min[0:1, 0:1], blob[0:1, 0:1], blob[0:1, 0:1],
                        OP.mult)
                nc.sync.dma_start(out=out_p[:], in_=o_min[:])

            gq = [0]                 # strict round-robin SWDGE queue counter
            qload = [0, 0, 0, 0]     # greedy byte-balancing across queues

            def pick_q(nch):
                q = qload.index(min(qload))
                qload[q] += nch
                return q
            gconst = None
            if mode == "gatcomp":
                gconst = cpool.tile([128, WD], BF16)
                nc.vector.memset(gconst[:], 0.01)

            # body repetition for precise marginal-time measurement (repN)
            for _rep in range(reps if mode != "min" else 0):
                if mode in ("gatbare", "gatdve", "gatdve2", "gatact", "gatpe"):
                    o_gb = xpool.tile([LAYERS[-1][1], S], F32, tag="xT")
                    nc.vector.memset(o_gb[:], 0.25)
                    gc2 = cpool.tile([128, WD], BF16, name="gc2")
                    nc.vector.memset(gc2[:], 0.01)
                    mc2 = cpool.tile([128, 128], BF16, name="mc2")
                    nc.vector.memset(mc2[:], 0.01)
                    for l in range(4):
                        fout = LAYERS[l][1]
                        for si, (j0, n_lo, n_hi, blocks) in enumerate(spans):
                            glo = wpool.tile([128, max(n_lo, 1) * WD], BF16,
                                             tag="glo", bufs=GBUFS)
                            ghi = wpool.tile([128, max(n_hi, 1) * WD], BF16,
                                             tag="ghi", bufs=GBUFS)
                            for h, gt, nch in ((0, glo, n_lo), (1, ghi, n_hi)):
                                if nch == 0:
                                    continue
                                i16 = (j0 + (0 if h == 0 else n_lo)) * 8
                                nc.gpsimd.dma_gather(
                                    gt[:, 0:nch * WD].rearrange(
                                        "p (c w) -> p c w", w=WD),
                                    tabx_p[:], idx16_sb[:, i16:i16 + nch * 8],
                                    nch * 128, nch * 128, WD,
                                    single_packet=False, queue_num=gq[0] % 4)
                                gq[0] += 1
                            if mode == "gatdve":
                                for b in blocks:
                                    for k, j in enumerate(block_cols[b]):
                                        m = wpool.tile([128, 128], BF16,
                                                       tag="m", bufs=16)
                                        nc.vector.tensor_scalar(
                                            m[:], iota_b, rl_sb[:, j:j + 1],
                                            norm_sb[:, j:j + 1],
                                            OP.is_equal, OP.mult)
                            elif mode == "gatdve2":
                                for b in blocks:
                                    for k, j in enumerate(block_cols[b]):
                                        m = ppool.tile([128, 128], BF16,
                                                       tag="mps", bufs=8)
                                        nc.vector.tensor_scalar(
                                            m[:], iota_b, rl_sb[:, j:j + 1],
                                            norm_sb[:, j:j + 1],
                                            OP.is_equal, OP.mult)
                            elif mode == "gatact":
                                for b in blocks:
                                    for k, j in enumerate(block_cols[b]):
                                        m = wpool.tile([128, 128], BF16,
                                                       tag="m", bufs=16)
                                        nc.scalar.activation(
                                            m[:], mc2[:],
                                            mybir.ActivationFunctionType.Copy)
                            elif mode == "gatpe":
                                for b in blocks:
                                    cols = block_cols[b]
                                    ps_a = ppool.tile([fout, 128], F32,
                                                      tag="psa", bufs=4)
                                    for k, j in enumerate(cols):
                                        nc.tensor.matmul(
                                            out=ps_a[:],
                                            lhsT=gc2[:, :fout], rhs=mc2[:],
                                            start=(k == 0),
                                            stop=(k == len(cols) - 1))
                    nc.vector.tensor_tensor(
                        o_gb[0:1, 0:1], ghi[0:1, 0:1], ghi[0:1, 0:1], OP.mult)
                    nc.sync.dma_start(out=out_p[:], in_=o_gb[:])
                    continue
                xT = xT0
                # prologue: h for layer 0 from the input xT0
                h_all = wpool.tile([128, NBH * WD], BF16, tag="hall", bufs=2)
                for b in range(NBH):
                    ps_h = ppool.tile([128, LAYERS[0][1]], F32, tag="ps_small")
                    nc.tensor.matmul(
                        out=ps_h[:], lhsT=xT0[:LAYERS[0][0],
                                             b * 128:(b + 1) * 128],
                        rhs=W_sb[0], start=True, stop=True)
                    nc.scalar.activation(
                        h_all[:, b * WD:b * WD + LAYERS[0][1]], ps_h[:],
                        AF.Copy)
                for l, (fin, fout) in enumerate(LAYERS):
                    hsh = dpool.tile([S, WD], BF16, name=f"hsh{l}_{_rep}")
                    nc.sync.dma_start(
                        out=hsh.rearrange("(b p) w -> p b w", p=128),
                        in_=h_all[:].rearrange("p (b w) -> p b w", w=WD))

                    htab = dpool.tile([NPAD, WD], BF16, addr_space="Shared",
                                      name=f"htab{l}_{_rep}")
                    if mode == "nocc":
                        nc.sync.dma_start(out=htab[0:S, :], in_=hsh[:])
                    else:
                        nc.gpsimd.collective_compute(
                            "AllGather", OP.bypass, replica_groups=AGRP,
                            ins=[hsh[:]], outs=[htab[:]])

                    # aggregation: agg^T[f, d] += G[e, f]^T @ onehot[e, d]
                    xT_next = xpool.tile([fout, S], BF16 if l + 1 < len(LAYERS)
                                         else F32, tag="xT", name=f"xT{l + 1}_sb")
                    h_next = (wpool.tile([128, NBH * WD], BF16, tag="hall",
                                         bufs=2, name=f"hnext{l}_{_rep}")
                              if l + 1 < len(LAYERS) else None)
                    if mode == "noagg":
                        nc.vector.memset(xT_next[:], 0.25)
                        xT = xT_next
                        continue
                    if mode in ("gatonly", "gatpar"):
                        nc.vector.memset(xT_next[:], 0.25)
                        for si, (j0, n_lo, n_hi, blocks) in enumerate(spans):
                            nch_s = n_lo + n_hi
                            g = wpool.tile([128, nch_s * WD], BF16, tag="g",
                                           bufs=GBUFS)
                            tlo = htab[0:SPLIT, :] if mode == "gatonly" \
                                else tabx_p[:]
                            thi = htab[SPLIT:NPAD, :] if mode == "gatonly" \
                                else tabx_p[:]
                            for h, tab, o0, nch in (
                                    (0, tlo, 0, n_lo),
                                    (1, thi, n_lo * WD, n_hi)):
                                if nch == 0:
                                    continue
                                i16 = (j0 + (0 if h == 0 else n_lo)) * 8
                                nc.gpsimd.dma_gather(
                                    g[:, o0:o0 + nch * WD].rearrange(
                                        "p (c w) -> p c w", w=WD),
                                    tab, idx16_sb[:, i16:i16 + nch * 8],
                                    nch * 128, nch * 128, WD,
                                    single_packet=False, queue_num=gq[0] % 4)
                                gq[0] += 1
                        xT = xT_next
                        continue
                    for si, (j0, n_lo, n_hi, blocks) in enumerate(spans):
                        glo = wpool.tile([128, max(n_lo, 1) * WD], BF16,
                                         tag="glo", bufs=GBUFS)
                        ghi = wpool.tile([128, max(n_hi, 1) * WD], BF16,
                                         tag="ghi", bufs=GBUFS)
                        for h, tab, gt, nch in ((0, htab[0:SPLIT, :], glo, n_lo),
                                                (1, htab[HI0:NPAD, :],
                                                 ghi, n_hi)):
                            if nch == 0:
                                continue
                            if mode == "nogat":
                                nc.vector.memset(gt[:, 0:1], 0.25)
                                continue
                            if mode == "gatcomp":
                                tab = tabx_p[:]
                            i16 = (j0 + (0 if h == 0 else n_lo)) * 8
                            nc.gpsimd.dma_gather(
                                gt[:, 0:nch * WD].rearrange(
                                    "p (c w) -> p c w", w=WD),
                                tab, idx16_sb[:, i16:i16 + nch * 8],
                                nch * 128, nch * 128, WD,
                                single_packet=False,
                                queue_num=0 if os.environ.get("GQ1")
                                else pick_q(nch))
                            gq[0] += 1
                        nch_s = n_lo + n_hi
                        m_span = wpool.tile([128, nch_s * DB], BF16,
                                            tag="m", bufs=3)
                        nc.sync.dma_start(
                            out=m_span[:],
                            in_=mtab_p[:, j0 * DB // 2:(j0 + nch_s) * DB // 2]
                            .bitcast(BF16))
                        for b in blocks:
                            cols = block_cols[b]
                            ps_a = ppool.tile([fout, DB], F32, tag="psa", bufs=4)
                            for k, j in enumerate(cols):
                                if mode == "gatcomp":
                                    gsrc, o0 = gconst, 0
                                elif j - j0 < n_lo:
                                    gsrc, o0 = glo, (j - j0) * WD
                                else:
                                    gsrc, o0 = ghi, (j - j0 - n_lo) * WD
                                nc.tensor.matmul(
                                    out=ps_a[:], lhsT=gsrc[:, o0:o0 + fout],
                                    rhs=m_span[:, (j - j0) * DB:
                                               (j - j0 + 1) * DB],
                                    start=(k == 0),
                                    stop=(k == len(cols) - 1))
                            # epilogue: +bias, ReLU (norm fully in M)
                            nc.scalar.activation(
                                xT_next[:, b * DB:(b + 1) * DB], ps_a[:],
                                AF.Relu if l + 1 < len(LAYERS) else AF.Identity,
                                bias=b_sb[l][:, 0:1])
                            if l + 1 < len(LAYERS):
                                # next layer's h for this block rides the agg
                                # phase (PE/Act have slack; hides the h phase
                                # and starts the collective earlier)
                                fin2, fout2 = LAYERS[l + 1]
                                ps_h = ppool.tile([128, fout2], F32,
                                                  tag="ps_small")
                                nc.tensor.matmul(
                                    out=ps_h[:],
                                    lhsT=xT_next[:fin2,
                                                 b * 128:(b + 1) * 128],
                                    rhs=W_sb[l + 1], start=True, stop=True)
                                nc.scalar.activation(
                                    h_next[:, b * WD:b * WD + fout2],
                                    ps_h[:], AF.Copy)
                    xT = xT_next
                    h_all = h_next

                nc.sync.dma_start(out=out_p[:], in_=xT[:])
    nc.finalize()   # Bacc: reg alloc + event-sem wait splitting
    return nc


def _make_in_maps(x, pre, W0, b0, W1, b1, W2, b2, W3, b3):
    CH_TOT, idx16, mtab, ids = (pre["CH_TOT"], pre["idx16"], pre["mtab"],
                                pre["ids"])
    rl_a, nc_a = pre["rl_a"], pre["nc_a"]
    off, ncols = _blob_offsets(CH_TOT)
    Ws = [np.asarray(W, np.float32).astype(BF) for W in (W0, W1, W2, W3)]
    bs = [np.asarray(b, np.float32).reshape(-1) for b in (b0, b1, b2, b3)]

    in_maps = []
    for c in range(NCORES):
        blob = np.zeros((128, ncols), np.float32)
        blob[:, off["idx16"]:off["idx16"] + CH_TOT * 4] = idx16[c].view(np.float32)
        blob[:, off["rl"]:off["rl"] + CH_TOT] = rl_a[c]
        blob[:, off["norm"]:off["norm"] + CH_TOT] = nc_a[c]
        blob[:, off["iota"]:off["iota"] + 64] = np.broadcast_to(
            np.arange(128, dtype=np.float32).astype(BF).view(np.float32)[None, :],
            (128, 64))
        xb = np.zeros((128, S), BF)
        valid = ids[c] >= 0
        xb[:, valid] = x[ids[c][valid]].T.astype(BF)
        blob[:, off["xT"]:off["xT"] + S // 2] = xb.view(np.float32)
        for l, (fin, fout) in enumerate(LAYERS):
            blob[:fin, off[f"W{l}"]:off[f"W{l}"] + fout // 2] = Ws[l].view(np.float32)
            blob[:fout, off[f"b{l}"]] = bs[l]
        in_maps.append({"blob": blob,
                        "mtab": mtab[c].view(np.float32)})
    return in_maps


def kernel(x, edge_index, edge_weight, W0, b0, W1, b1, W2, b2, W3, b3):
    global LAST_RESULTS
    x = np.ascontiguousarray(np.asarray(x, np.float32))
    pre = _preprocess(np.asarray(edge_index), np.asarray(edge_weight))

    nc = _build(pre)

    in_maps = _make_in_maps(x, pre, W0, b0, W1, b1, W2, b2, W3, b3)

    res = run_bass_kernel_spmd(nc, in_maps, core_ids=list(range(NCORES)),
                               trace=TRACE, trace_kwargs=dict(TRACE_KW))
    LAST_RESULTS = res
    ids = pre["ids"]
    out = np.zeros((N_NODES, LAYERS[-1][1]), np.float32)
    for c in range(NCORES):
        valid = ids[c] >= 0
        out[ids[c][valid]] = res.results[c]["out"][:, valid].T
    return np.ascontiguousarray(out)


# revision 42
# speedup vs baseline: 1.1854x; 1.0949x over previous
"""Trainium2 Bass kernel for a 4-layer GCN (nn_GCNModel), SPMD across 8 NeuronCores.

Strategy (graph/data parallel per the sharding hint):
  - Nodes are partitioned across the 8 cores (6250 real rows/core, padded to
    6272 = 49 blocks of 128).  Each core owns the edges whose DESTINATION
    falls in its shard (plus that shard's self-loops), pre-sorted by
    destination block on the host and padded with zero-weight edges so every
    core sees the same per-block chunk structure (SPMD: one NEFF, 8 cores).
  - Nodes are PERMUTED on the host (round-robin over (core, dest-block)
    bins by descending in-degree) so every bin carries ~equal edge load;
    the lo/hi gather tables OVERLAP (hi base 17408) so boundary edges can
    go to either half, rounding each block's lo half to exactly full
    128-edge chunks (zero lo padding).
  - GCN normalization is computed entirely on the HOST: the full per-edge
    coefficient dis[row]*w*dis[col] is baked into precomputed one-hot M
    tiles (M[e,d] = (row_local[e]==d)*norm[e], bf16) that are STREAMED from
    DRAM per span. Building M on DVE/Act is a trap: any compute-engine
    SBUF-write traffic runs mutually exclusive with the gather DMA's SBUF
    writes and serializes the kernel; DMA-streamed M overlaps fine.
  - Per layer: each core computes h = X_shard @ W for its rows (bf16), the
    padded h table is AllGather'd to every core's DRAM, then each core
    gathers the source rows of its edges with `dma_gather` (128 rows per
    chunk; int16 indices force the lo/hi table split; separate lo/hi
    g-tiles, ~2.4k-3.2k rows per gather, greedy byte-balanced over the 4
    SWDGE queues, 8 buffers deep so all queues stay busy) and scatter-adds
    via TensorE matmul accumulation in PSUM:
    agg^T[f,d] += G[e,f]^T @ M[e,d].
  - agg^T comes out feature-major [F, nodes] — exactly the stationary
    operand layout the next layer's matmul wants; bias+ReLU ride the
    PSUM->SBUF copy on the Scalar engine (bias is per-partition there).
  - All per-core constants ship as ONE packed f32 blob (single DMA/sem) and
    each h-table store is a single DMA — TRN2 instructions carry one sync
    wait, so fan-in of DMA semaphore lanes must stay small (Bacc splits the
    rest via event semaphores).

Self-contained: hardcodes all shapes; host side only computes norms,
reorders/pads indices and concatenates shard outputs.

Known remaining headroom (untried/unfinished):
  - M tiles are mostly zeros: sorting each (block, half)'s edges by dest and
    cutting chunks at fixed 16-dest (lo) / 8-dest (hi) ranges would let M
    stream ~8x fewer bytes (psum sub-range writes with start=True on the
    covering lo chunk, start=False accumulation for hi), trading ~+30%
    gather padding -- est. -250us at the ~200 GB/s SBUF DMA-write wall.
  - The per-layer AllGather (~40us/layer) is serial; chunked collectives
    could hide most of it.
"""

import os
import sys

import numpy as np
import ml_dtypes

for _p in ("/opt/trn_rl_repo", "/root/.axon_site/_ro/trn_rl_repo"):
    if os.path.isdir(_p) and _p not in sys.path:
        sys.path.append(_p)

from concourse import bacc, mybir, tile  # noqa: E402
from concourse.bass_utils import run_bass_kernel_spmd  # noqa: E402

F32 = mybir.dt.float32
BF16 = mybir.dt.bfloat16
I16 = mybir.dt.int16
BF = ml_dtypes.bfloat16

N_NODES = 50000
NCORES = 8
S_REAL = N_NODES // NCORES           # 6250 real nodes per core
NBH = (S_REAL + 127) // 128          # 49 h/source blocks (128 nodes) per core
S = NBH * 128                        # 6272 padded shard size
DB = 128                             # dest-block width (M tile columns)
NBD = S // DB                        # 98 dest blocks per core
NPAD = NCORES * S                    # 50176 padded global nodes
SPLIT = 32768                        # int16 index limit for dma_gather
HI0 = NCORES * ((N_NODES // NCORES + 127) // 128 * 128) - 32768
#     = 17408: hi table base (overlaps lo: flexible
                                     # edges with src in [HI0, SPLIT) can go
                                     # to either half -> round lo to full
                                     # chunks, zero lo padding)
SPAN = 2                             # dest blocks per gather instruction
WD = 128                             # h-table row width (bf16 -> 256B rows)
MAX_IDX_PER_GATHER = 8192            # HW-validated dma_gather size
GBUFS = 8                            # gather buffers in flight
# (F_in, F_out) per layer
LAYERS = [(128, 100), (100, 100), (100, 50), (50, 6)]

TRACE = False
TRACE_KW = {}
LAST_RESULTS = None


def _wrap_idx16(flat):
    """dma_gather index layout: idx i at (partition i%16, column i//16),
    replicated to all 8 16-partition groups."""
    n = len(flat)
    assert n % 16 == 0
    w = flat.reshape(n // 16, 16).T.astype(np.int16)     # [16, n/16]
    return np.tile(w, (8, 1))                            # [128, n/16]


def _preprocess(edge_index, edge_weight):
    """Shard + sort edges by (dest block, lo/hi col half); pad each half to a
    multiple of 128 chunks with SPMD-uniform counts; group chunk columns by
    gather span: [span lo parts (by block) | span hi parts (by block)].

    Host also computes the full GCN normalization: dis = rsqrt(deg), per-edge
    normcol = dis[row]*w*dis[col] (the complete coefficient, folded into the
    one-hot M matrices on device -- no epilogue multiply needed).

    Returns (spans, block_cols, CH_TOT, idx16, rl_a, nc_a):
      spans: list of (j0, n_lo_chunks, n_hi_chunks, [blocks])
      block_cols: per block, the ordered chunk-column indices
    """
    row = edge_index[0].astype(np.int64)
    col = edge_index[1].astype(np.int64)
    w = edge_weight.astype(np.float32)
    loop = np.arange(N_NODES, dtype=np.int64)
    row = np.concatenate([row, loop])
    col = np.concatenate([col, loop])
    w = np.concatenate([w, np.ones(N_NODES, np.float32)])

    deg = np.zeros(N_NODES, np.float64)
    np.add.at(deg, row, w.astype(np.float64))
    dis_n = np.where(deg > 0, 1.0 / np.sqrt(np.maximum(deg, 1e-12)), 0.0)
    dis_n = dis_n.astype(np.float32)
    normcol = dis_n[row] * w * dis_n[col]                # full edge norm

    # Node permutation: deal nodes round-robin over the 8*NBD (core, block)
    # bins in descending in-degree order, so every bin carries ~equal edge
    # load (kills the max-over-cores chunk padding).
    edeg = np.ones(N_NODES, np.int64)
    np.add.at(edeg, edge_index[0].astype(np.int64), 1)
    rank = np.argsort(-edeg, kind="stable")
    NBINS = NCORES * NBD
    pos = np.empty(N_NODES, np.int64)        # node -> global padded position
    bin_of = np.arange(N_NODES) % NBINS
    slot_of = np.arange(N_NODES) // NBINS
    cores_of = bin_of % NCORES
    blk_of = bin_of // NCORES
    pos[rank] = cores_of * S + blk_of * DB + slot_of
    assert slot_of.max() < DB
    ids = np.full((NCORES, S), -1, np.int64)  # (core, local pos) -> node
    ids[pos // S, pos % S] = np.arange(N_NODES)

    posr = pos[row]
    core = posr // S
    r_in = posr % S
    blk = r_in // DB
    rloc = (r_in % DB).astype(np.float32)
    colp = pos[col]

    # Flexible lo/hi assignment: within each (core, block), rank edges by
    # source position; the first 128*K_b go lo (K_b = max over cores of
    # ceil(#forced_lo/128), forced_lo = src < HI0), the rest hi.  All lo
    # chunks are exactly full; only hi carries padding.
    grp = core * NBD + blk
    order0 = np.lexsort((colp, grp))
    gsort = grp[order0]
    starts = np.searchsorted(gsort, np.arange(NCORES * NBD))
    rank = np.empty(len(grp), np.int64)
    rank[order0] = np.arange(len(grp)) - starts[gsort]

    counts_flo = np.zeros((NCORES, NBD), np.int64)
    np.add.at(counts_flo, (core[colp < HI0], blk[colp < HI0]), 1)
    K_b = -(-counts_flo.max(axis=0) // 128)              # [NBD] lo chunks
    counts_tot = np.zeros((NCORES, NBD), np.int64)
    np.add.at(counts_tot, (core, blk), 1)
    assert (counts_tot >= 128 * K_b[None, :]).all()

    hi = (rank >= 128 * K_b[blk]).astype(np.int64)
    # hi-assigned edges must lie in the hi table's range
    assert (colp[hi == 1] >= HI0).all()
    assert (colp[hi == 0] < SPLIT).all()

    CHP = np.stack([K_b, -(-np.maximum(
        counts_tot - 128 * K_b[None, :], 0).max(axis=0) // 128)], axis=1)
    CH_TOT = int(CHP.sum())

    # span/chunk-column layout
    spans = []
    block_cols = [[] for _ in range(NBD)]
    part_col = {}                                        # (b, h) -> start col
    j = 0
    for s0 in range(0, NBD, SPAN):
        blocks = list(range(s0, min(s0 + SPAN, NBD)))
        j0 = j
        n_lo = n_hi = 0
        for h in range(2):
            for b in blocks:
                nch = int(CHP[b, h])
                part_col[(b, h)] = j
                block_cols[b].extend(range(j, j + nch))
                j += nch
                if h == 0:
                    n_lo += nch
                else:
                    n_hi += nch
        assert n_lo * 128 <= MAX_IDX_PER_GATHER
        assert n_hi * 128 <= MAX_IDX_PER_GATHER
        spans.append((j0, n_lo, n_hi, blocks))
    assert j == CH_TOT

    idx16 = np.zeros((NCORES, 128, CH_TOT * 8), np.int16)
    rl_a = np.zeros((NCORES, 128, CH_TOT), np.float32)
    nc_a = np.zeros((NCORES, 128, CH_TOT), np.float32)
    mtab = np.zeros((NCORES, 128, CH_TOT * DB), BF)

    order = np.lexsort((hi, blk, core))
    scolp, srloc, snc = colp[order], rloc[order], normcol[order]
    score, sblk, shi = core[order], blk[order], hi[order]
    bounds = np.searchsorted(
        score * (NBD * 2) + sblk * 2 + shi, np.arange(NCORES * NBD * 2 + 1))

    for c in range(NCORES):
        for b in range(NBD):
            for h in range(2):
                k = (c * NBD + b) * 2 + h
                s0, s1 = bounds[k], bounds[k + 1]
                nch = int(CHP[b, h])
                if nch == 0:
                    assert s1 == s0
                    continue
                n = nch * 128
                jc = part_col[(b, h)]
                lim = SPLIT if h == 0 else NPAD - HI0
                # spread padding indices so they don't hammer row 0
                ii = (np.arange(n, dtype=np.int64) * 97 + jc * 131) % lim
                ii[:s1 - s0] = scolp[s0:s1] - h * HI0
                rr = np.zeros(n, np.float32)
                rr[:s1 - s0] = srloc[s0:s1]
                nn = np.zeros(n, np.float32)
                nn[:s1 - s0] = snc[s0:s1]
                rl_a[c, :, jc:jc + nch] = rr.reshape(nch, 128).T
                nc_a[c, :, jc:jc + nch] = nn.reshape(nch, 128).T
                idx16[c, :, jc * 8:jc * 8 + n // 16] = _wrap_idx16(ii)
    # M tiles (layer-independent): mtab[:, j*DB:(j+1)*DB] = one-hot(rl)*norm
    dDB = np.arange(DB, dtype=np.float32)[None, None, :]
    for c in range(NCORES):
        m = (rl_a[c][:, :, None] == dDB) * nc_a[c][:, :, None]
        mtab[c] = m.astype(BF).reshape(128, CH_TOT * DB)
    return dict(spans=spans, block_cols=block_cols, CH_TOT=CH_TOT,
                idx16=idx16, rl_a=rl_a, nc_a=nc_a, mtab=mtab, ids=ids)


def _blob_offsets(CH_TOT):
    """Column layout of the packed per-core constant blob [128, NCOLS] f32."""
    off = {}
    o = 0
    for k, n in (("idx16", CH_TOT * 4), ("xT", S // 2)):
        off[k] = o
        o += n
    for l, (fin, fout) in enumerate(LAYERS):
        off[f"W{l}"] = o
        o += fout // 2
    for l, (fin, fout) in enumerate(LAYERS):
        off[f"b{l}"] = o
        o += 1
    return off, o


def _build(pre, mode="full"):
    spans = pre["spans"]
    block_cols = pre["block_cols"]
    CH_TOT = pre["CH_TOT"]
    # mode: "<base>[-repN]" where base is one of
    #   full  - real kernel
    #   nodeg - alias of full (deg phase is host-side now)
    #   nocc  - collectives -> local DMA (wrong numerics; timing ablation)
    #   noagg - skip gather+aggregation entirely (ablation)
    #   nogat - skip the dma_gather instructions only (ablation)
    #   min   - minimal output path
    # repN repeats the whole body N times for marginal-time measurement.
    reps = 1
    if "-rep" in mode:
        mode, _, r = mode.partition("-rep")
        reps = int(r)
    elif mode.startswith("rep"):
        reps, mode = int(mode[3:]), "full"
    if mode == "nodeg":
        mode = "full"
    nc = bacc.Bacc(None, num_devices=NCORES, num_swdge_queues=4)
    AGRP = [list(range(NCORES))]
    AF = mybir.ActivationFunctionType
    OP = mybir.AluOpType

    off, ncols = _blob_offsets(CH_TOT)
    blob_p = nc.declare_dram_parameter("blob", [128, ncols], F32, isOutput=False)
    out_p = nc.declare_dram_parameter("out", [LAYERS[-1][1], S], F32, isOutput=True)
    tabx_p = (nc.declare_dram_parameter("tabx", [SPLIT, WD], BF16,
                                        isOutput=False)
              if mode.startswith("gat") and mode != "gatonly" else None)
    mtab_p = nc.declare_dram_parameter("mtab", [128, CH_TOT * DB // 2], F32,
                                       isOutput=False)

    with tile.TileContext(nc, num_cores=NCORES) as tc:
        with (
            tc.tile_pool(name="const", bufs=1) as cpool,
            tc.tile_pool(name="xpool", bufs=2) as xpool,
            tc.tile_pool(name="dram", bufs=1, space="DRAM") as dpool,
            tc.tile_pool(name="psum", bufs=2, space="PSUM") as ppool,
            tc.tile_pool(name="work", bufs=3) as wpool,
        ):
            blob = cpool.tile([128, ncols], F32)
            nc.sync.dma_start(out=blob[:], in_=blob_p[:])

            idx16_sb = blob[:, off["idx16"]:off["idx16"] + CH_TOT * 4].bitcast(I16)
            xT0 = blob[:, off["xT"]:off["xT"] + S // 2].bitcast(BF16)
            W_sb = [blob[:LAYERS[l][0],
                         off[f"W{l}"]:off[f"W{l}"] + LAYERS[l][1] // 2].bitcast(BF16)
                    for l in range(4)]
            b_sb = [blob[:LAYERS[l][1], off[f"b{l}"]:off[f"b{l}"] + 1]
                    for l in range(4)]

            if mode == "min":
                o_min = xpool.tile([LAYERS[-1][1], S], F32, tag="xT")
                nc.vector.memset(o_min[:], 0.25)
                for _rep in range(reps):
                    nc.vector.tensor_tensor(
                        o_min[0:1, 0:1], blob[0:1, 0:1], blob[0:1, 0:1],
                        OP.mult)
                nc.sync.dma_start(out=out_p[:], in_=o_min[:])

            gq = [0]                 # strict round-robin SWDGE queue counter
            qload = [0, 0, 0, 0]     # greedy byte-balancing across queues

            def pick_q(nch):
                q = qload.index(min(qload))
                qload[q] += nch
                return q
            gconst = None
            if mode == "gatcomp":
                gconst = cpool.tile([128, WD], BF16)
                nc.vector.memset(gconst[:], 0.01)

            # body repetition for precise marginal-time measurement (repN)
            for _rep in range(reps if mode != "min" else 0):
                if mode in ("gatbare", "gatdve", "gatdve2", "gatact", "gatpe"):
                    o_gb = xpool.tile([LAYERS[-1][1], S], F32, tag="xT")
                    nc.vector.memset(o_gb[:], 0.25)
                    gc2 = cpool.tile([128, WD], BF16, name="gc2")
                    nc.vector.memset(gc2[:], 0.01)
                    mc2 = cpool.tile([128, 128], BF16, name="mc2")
                    nc.vector.memset(mc2[:], 0.01)
                    for l in range(4):
                        fout = LAYERS[l][1]
                        for si, (j0, n_lo, n_hi, blocks) in enumerate(spans):
                            glo = wpool.tile([128, max(n_lo, 1) * WD], BF16,
                                             tag="glo", bufs=GBUFS)
                            ghi = wpool.tile([128, max(n_hi, 1) * WD], BF16,
                                             tag="ghi", bufs=GBUFS)
                            for h, gt, nch in ((0, glo, n_lo), (1, ghi, n_hi)):
                                if nch == 0:
                                    continue
                                i16 = (j0 + (0 if h == 0 else n_lo)) * 8
                                nc.gpsimd.dma_gather(
                                    gt[:, 0:nch * WD].rearrange(
                                        "p (c w) -> p c w", w=WD),
                                    tabx_p[:], idx16_sb[:, i16:i16 + nch * 8],
                                    nch * 128, nch * 128, WD,
                                    single_packet=False, queue_num=gq[0] % 4)
                                gq[0] += 1
                            if mode == "gatact":
                                for b in blocks:
                                    for k, j in enumerate(block_cols[b]):
                                        m = wpool.tile([128, 128], BF16,
                                                       tag="m", bufs=16)
                                        nc.scalar.activation(
                                            m[:], mc2[:],
                                            mybir.ActivationFunctionType.Copy)
                            elif mode == "gatpe":
                                for b in blocks:
                                    cols = block_cols[b]
                                    ps_a = ppool.tile([fout, 128], F32,
                                                      tag="psa", bufs=4)
                                    for k, j in enumerate(cols):
                                        nc.tensor.matmul(
                                            out=ps_a[:],
                                            lhsT=gc2[:, :fout], rhs=mc2[:],
                                            start=(k == 0),
                                            stop=(k == len(cols) - 1))
                    nc.vector.tensor_tensor(
                        o_gb[0:1, 0:1], ghi[0:1, 0:1], ghi[0:1, 0:1], OP.mult)
                    nc.sync.dma_start(out=out_p[:], in_=o_gb[:])
                    continue
                xT = xT0
                # prologue: h for layer 0 from the input xT0
                h_all = wpool.tile([128, NBH * WD], BF16, tag="hall", bufs=2)
                for b in range(NBH):
                    ps_h = ppool.tile([128, LAYERS[0][1]], F32, tag="ps_small")
                    nc.tensor.matmul(
                        out=ps_h[:], lhsT=xT0[:LAYERS[0][0],
                                             b * 128:(b + 1) * 128],
                        rhs=W_sb[0], start=True, stop=True)
                    nc.scalar.activation(
                        h_all[:, b * WD:b * WD + LAYERS[0][1]], ps_h[:],
                        AF.Copy)
                for l, (fin, fout) in enumerate(LAYERS):
                    hsh = dpool.tile([S, WD], BF16, name=f"hsh{l}_{_rep}")
                    nc.sync.dma_start(
                        out=hsh.rearrange("(b p) w -> p b w", p=128),
                        in_=h_all[:].rearrange("p (b w) -> p b w", w=WD))

                    htab = dpool.tile([NPAD, WD], BF16, addr_space="Shared",
                                      name=f"htab{l}_{_rep}")
                    if mode == "nocc":
                        nc.sync.dma_start(out=htab[0:S, :], in_=hsh[:])
                    else:
                        nc.gpsimd.collective_compute(
                            "AllGather", OP.bypass, replica_groups=AGRP,
                            ins=[hsh[:]], outs=[htab[:]])

                    # aggregation: agg^T[f, d] += G[e, f]^T @ onehot[e, d]
                    xT_next = xpool.tile([fout, S], BF16 if l + 1 < len(LAYERS)
                                         else F32, tag="xT", name=f"xT{l + 1}_sb")
                    h_next = (wpool.tile([128, NBH * WD], BF16, tag="hall",
                                         bufs=2, name=f"hnext{l}_{_rep}")
                              if l + 1 < len(LAYERS) else None)
                    if mode == "noagg":
                        nc.vector.memset(xT_next[:], 0.25)
                        xT = xT_next
                        continue
                    if mode in ("gatonly", "gatpar"):
                        nc.vector.memset(xT_next[:], 0.25)
                        for si, (j0, n_lo, n_hi, blocks) in enumerate(spans):
                            nch_s = n_lo + n_hi
                            g = wpool.tile([128, nch_s * WD], BF16, tag="g",
                                           bufs=GBUFS)
                            tlo = htab[0:SPLIT, :] if mode == "gatonly" \
                                else tabx_p[:]
                            thi = htab[SPLIT:NPAD, :] if mode == "gatonly" \
                                else tabx_p[:]
                            for h, tab, o0, nch in (
                                    (0, tlo, 0, n_lo),
                                    (1, thi, n_lo * WD, n_hi)):
                                if nch == 0:
                                    continue
                                i16 = (j0 + (0 if h == 0 else n_lo)) * 8
                                nc.gpsimd.dma_gather(
                                    g[:, o0:o0 + nch * WD].rearrange(
                                        "p (c w) -> p c w", w=WD),
                                    tab, idx16_sb[:, i16:i16 + nch * 8],
                                    nch * 128, nch * 128, WD,
                                    single_packet=False, queue_num=gq[0] % 4)
                                gq[0] += 1
                        xT = xT_next
                        continue
                    for si, (j0, n_lo, n_hi, blocks) in enumerate(spans):
                        glo = wpool.tile([128, max(n_lo, 1) * WD], BF16,
                                         tag="glo", bufs=GBUFS + 2)
                        ghi = wpool.tile([128, max(n_hi, 1) * WD], BF16,
                                         tag="ghi", bufs=GBUFS)
                        for h, tab, gt, nch in ((0, htab[0:SPLIT, :], glo, n_lo),
                                                (1, htab[HI0:NPAD, :],
                                                 ghi, n_hi)):
                            if nch == 0:
                                continue
                            if mode == "nogat":
                                nc.vector.memset(gt[:, 0:1], 0.25)
                                continue
                            if mode == "gatcomp":
                                tab = tabx_p[:]
                            i16 = (j0 + (0 if h == 0 else n_lo)) * 8
                            nc.gpsimd.dma_gather(
                                gt[:, 0:nch * WD].rearrange(
                                    "p (c w) -> p c w", w=WD),
                                tab, idx16_sb[:, i16:i16 + nch * 8],
                                nch * 128, nch * 128, WD,
                                single_packet=False,
                                queue_num=0 if os.environ.get("GQ1")
                                else pick_q(nch))
                            gq[0] += 1
                        nch_s = n_lo + n_hi
                        m_span = wpool.tile([128, nch_s * DB], BF16,
                                            tag="m", bufs=3)
                        nc.sync.dma_start(
                            out=m_span[:],
                            in_=mtab_p[:, j0 * DB // 2:(j0 + nch_s) * DB // 2]
                            .bitcast(BF16))
                        for b in blocks:
                            cols = block_cols[b]
                            ps_a = ppool.tile([fout, DB], F32, tag="psa", bufs=4)
                            for k, j in enumerate(cols):
                                if mode == "gatcomp":
                                    gsrc, o0 = gconst, 0
                                elif j - j0 < n_lo:
                                    gsrc, o0 = glo, (j - j0) * WD
                                else:
                                    gsrc, o0 = ghi, (j - j0 - n_lo) * WD
                                nc.tensor.matmul(
                                    out=ps_a[:], lhsT=gsrc[:, o0:o0 + fout],
                                    rhs=m_span[:, (j - j0) * DB:
                                               (j - j0 + 1) * DB],
                                    start=(k == 0),
                                    stop=(k == len(cols) - 1))
                            # epilogue: +bias, ReLU (norm fully in M)
                            nc.scalar.activation(
                                xT_next[:, b * DB:(b + 1) * DB], ps_a[:],
                                AF.Relu if l + 1 < len(LAYERS) else AF.Identity,
                                bias=b_sb[l][:, 0:1])
                            if l + 1 < len(LAYERS):
                                # next layer's h for this block rides the agg
                                # phase (PE/Act have slack; hides the h phase
                                # and starts the collective earlier)
                                fin2, fout2 = LAYERS[l + 1]
                                ps_h = ppool.tile([128, fout2], F32,
                                                  tag="ps_small")
                                nc.tensor.matmul(
                                    out=ps_h[:],
                                    lhsT=xT_next[:fin2,
                                                 b * 128:(b + 1) * 128],
                                    rhs=W_sb[l + 1], start=True, stop=True)
                                nc.scalar.activation(
                                    h_next[:, b * WD:b * WD + fout2],
                                    ps_h[:], AF.Copy)
                    xT = xT_next
                    h_all = h_next

                nc.sync.dma_start(out=out_p[:], in_=xT[:])
    nc.finalize()   # Bacc: reg alloc + event-sem wait splitting
    return nc


def _make_in_maps(x, pre, W0, b0, W1, b1, W2, b2, W3, b3):
    CH_TOT, idx16, mtab, ids = (pre["CH_TOT"], pre["idx16"], pre["mtab"],
                                pre["ids"])
    rl_a, nc_a = pre["rl_a"], pre["nc_a"]
    off, ncols = _blob_offsets(CH_TOT)
    Ws = [np.asarray(W, np.float32).astype(BF) for W in (W0, W1, W2, W3)]
    bs = [np.asarray(b, np.float32).reshape(-1) for b in (b0, b1, b2, b3)]

    in_maps = []
    for c in range(NCORES):
        blob = np.zeros((128, ncols), np.float32)
        blob[:, off["idx16"]:off["idx16"] + CH_TOT * 4] = idx16[c].view(np.float32)
        xb = np.zeros((128, S), BF)
        valid = ids[c] >= 0
        xb[:, valid] = x[ids[c][valid]].T.astype(BF)
        blob[:, off["xT"]:off["xT"] + S // 2] = xb.view(np.float32)
        for l, (fin, fout) in enumerate(LAYERS):
            blob[:fin, off[f"W{l}"]:off[f"W{l}"] + fout // 2] = Ws[l].view(np.float32)
            blob[:fout, off[f"b{l}"]] = bs[l]
        in_maps.append({"blob": blob,
                        "mtab": mtab[c].view(np.float32)})
    return in_maps


def kernel(x, edge_index, edge_weight, W0, b0, W1, b1, W2, b2, W3, b3):
    global LAST_RESULTS
    x = np.ascontiguousarray(np.asarray(x, np.float32))
    pre = _preprocess(np.asarray(edge_index), np.asarray(edge_weight))

    nc = _build(pre)

    in_maps = _make_in_maps(x, pre, W0, b0, W1, b1, W2, b2, W3, b3)

    res = run_bass_kernel_spmd(nc, in_maps, core_ids=list(range(NCORES)),
                               trace=TRACE, trace_kwargs=dict(TRACE_KW))
    LAST_RESULTS = res
    ids = pre["ids"]
    out = np.zeros((N_NODES, LAYERS[-1][1]), np.float32)
    for c in range(NCORES):
        valid = ids[c] >= 0
        out[ids[c][valid]] = res.results[c]["out"][:, valid].T
    return np.ascontiguousarray(out)


# revision 45
# speedup vs baseline: 1.2643x; 1.0666x over previous
"""Trainium2 Bass kernel for a 4-layer GCN (nn_GCNModel), SPMD across 8 NeuronCores.

Strategy (graph/data parallel per the sharding hint):
  - Nodes are partitioned across the 8 cores (6250 real rows/core, padded to
    6272 = 49 blocks of 128).  Each core owns the edges whose DESTINATION
    falls in its shard (plus that shard's self-loops), pre-sorted by
    destination block on the host and padded with zero-weight edges so every
    core sees the same per-block chunk structure (SPMD: one NEFF, 8 cores).
  - Nodes are PERMUTED on the host (round-robin over (core, dest-block)
    bins by descending in-degree) so every bin carries ~equal edge load;
    the lo/hi gather tables OVERLAP (hi base 17408) so boundary edges can
    go to either half, rounding each block's lo half to exactly full
    128-edge chunks (zero lo padding).
  - GCN normalization is computed entirely on the HOST: the full per-edge
    coefficient dis[row]*w*dis[col] is baked into precomputed one-hot M
    tiles (M[e,d] = (row_local[e]==d)*norm[e], bf16) that are STREAMED from
    DRAM per span. Building M on DVE/Act is a trap: any compute-engine
    SBUF-write traffic runs mutually exclusive with the gather DMA's SBUF
    writes and serializes the kernel; DMA-streamed M overlaps fine.
  - Per layer: each core computes h = X_shard @ W for its rows (bf16), the
    padded h table is AllGather'd to every core's DRAM, then each core
    gathers the source rows of its edges with `dma_gather` (128 rows per
    chunk; int16 indices force the lo/hi table split; separate lo/hi
    g-tiles, ~2.4k-3.2k rows per gather, greedy byte-balanced over the 4
    SWDGE queues, 8 buffers deep so all queues stay busy) and scatter-adds
    via TensorE matmul accumulation in PSUM:
    agg^T[f,d] += G[e,f]^T @ M[e,d].
  - agg^T comes out feature-major [F, nodes] — exactly the stationary
    operand layout the next layer's matmul wants; bias+ReLU ride the
    PSUM->SBUF copy on the Scalar engine (bias is per-partition there).
  - All per-core constants ship as ONE packed f32 blob (single DMA/sem) and
    each h-table store is a single DMA — TRN2 instructions carry one sync
    wait, so fan-in of DMA semaphore lanes must stay small (Bacc splits the
    rest via event semaphores).

Self-contained: hardcodes all shapes; host side only computes norms,
reorders/pads indices and concatenates shard outputs.

Known remaining headroom (untried/unfinished):
  - M tiles are mostly zeros: sorting each (block, half)'s edges by dest and
    cutting chunks at fixed 16-dest (lo) / 8-dest (hi) ranges would let M
    stream ~8x fewer bytes (psum sub-range writes with start=True on the
    covering lo chunk, start=False accumulation for hi), trading ~+30%
    gather padding -- est. -250us at the ~200 GB/s SBUF DMA-write wall.
  - The per-layer AllGather (~40us/layer) is serial; chunked collectives
    could hide most of it.
"""

import os
import sys

import numpy as np
import ml_dtypes

for _p in ("/opt/trn_rl_repo", "/root/.axon_site/_ro/trn_rl_repo"):
    if os.path.isdir(_p) and _p not in sys.path:
        sys.path.append(_p)

from concourse import bacc, mybir, tile  # noqa: E402
from concourse.bass_utils import run_bass_kernel_spmd  # noqa: E402

F32 = mybir.dt.float32
BF16 = mybir.dt.bfloat16
I16 = mybir.dt.int16
BF = ml_dtypes.bfloat16

N_NODES = 50000
NCORES = 8
S_REAL = N_NODES // NCORES           # 6250 real nodes per core
NBH = (S_REAL + 127) // 128          # 49 h/source blocks (128 nodes) per core
S = NBH * 128                        # 6272 padded shard size
DB = 128                             # dest-block width (M tile columns)
NBD = S // DB                        # 98 dest blocks per core
NPAD = NCORES * S                    # 50176 padded global nodes
SPLIT = 32768                        # int16 index limit for dma_gather
HI0 = NCORES * ((N_NODES // NCORES + 127) // 128 * 128) - 32768
#     = 17408: hi table base (overlaps lo: flexible
                                     # edges with src in [HI0, SPLIT) can go
                                     # to either half -> round lo to full
                                     # chunks, zero lo padding)
SPAN = 2                             # dest blocks per gather instruction
WD = 128                             # h-table row width (bf16 -> 256B rows)
MAX_IDX_PER_GATHER = 8192            # HW-validated dma_gather size
GBUFS = 8                            # gather buffers in flight
# (F_in, F_out) per layer
LAYERS = [(128, 100), (100, 100), (100, 50), (50, 6)]

TRACE = False
TRACE_KW = {}
LAST_RESULTS = None


def _wrap_idx16(flat):
    """dma_gather index layout: idx i at (partition i%16, column i//16),
    replicated to all 8 16-partition groups."""
    n = len(flat)
    assert n % 16 == 0
    w = flat.reshape(n // 16, 16).T.astype(np.int16)     # [16, n/16]
    return np.tile(w, (8, 1))                            # [128, n/16]


def _preprocess(edge_index, edge_weight):
    """Shard + sort edges by (dest block, lo/hi col half); pad each half to a
    multiple of 128 chunks with SPMD-uniform counts; group chunk columns by
    gather span: [span lo parts (by block) | span hi parts (by block)].

    Host also computes the full GCN normalization: dis = rsqrt(deg), per-edge
    normcol = dis[row]*w*dis[col] (the complete coefficient, folded into the
    one-hot M matrices on device -- no epilogue multiply needed).

    Returns (spans, block_cols, CH_TOT, idx16, rl_a, nc_a):
      spans: list of (j0, n_lo_chunks, n_hi_chunks, [blocks])
      block_cols: per block, the ordered chunk-column indices
    """
    row = edge_index[0].astype(np.int64)
    col = edge_index[1].astype(np.int64)
    w = edge_weight.astype(np.float32)
    loop = np.arange(N_NODES, dtype=np.int64)
    row = np.concatenate([row, loop])
    col = np.concatenate([col, loop])
    w = np.concatenate([w, np.ones(N_NODES, np.float32)])

    deg = np.zeros(N_NODES, np.float64)
    np.add.at(deg, row, w.astype(np.float64))
    dis_n = np.where(deg > 0, 1.0 / np.sqrt(np.maximum(deg, 1e-12)), 0.0)
    dis_n = dis_n.astype(np.float32)
    normcol = dis_n[row] * w * dis_n[col]                # full edge norm

    # Node permutation: deal nodes round-robin over the 8*NBD (core, block)
    # bins in descending in-degree order, so every bin carries ~equal edge
    # load (kills the max-over-cores chunk padding).
    edeg = np.ones(N_NODES, np.int64)
    np.add.at(edeg, edge_index[0].astype(np.int64), 1)
    rank = np.argsort(-edeg, kind="stable")
    NBINS = NCORES * NBD
    pos = np.empty(N_NODES, np.int64)        # node -> global padded position
    bin_of = np.arange(N_NODES) % NBINS
    slot_of = np.arange(N_NODES) // NBINS
    cores_of = bin_of % NCORES
    blk_of = bin_of // NCORES
    pos[rank] = cores_of * S + blk_of * DB + slot_of
    assert slot_of.max() < DB
    ids = np.full((NCORES, S), -1, np.int64)  # (core, local pos) -> node
    ids[pos // S, pos % S] = np.arange(N_NODES)

    posr = pos[row]
    core = posr // S
    r_in = posr % S
    blk = r_in // DB
    rloc = (r_in % DB).astype(np.float32)
    colp = pos[col]

    # Flexible lo/hi assignment: within each (core, block), rank edges by
    # source position; the first 128*K_b go lo (K_b = max over cores of
    # ceil(#forced_lo/128), forced_lo = src < HI0), the rest hi.  All lo
    # chunks are exactly full; only hi carries padding.
    grp = core * NBD + blk
    order0 = np.lexsort((colp, grp))
    gsort = grp[order0]
    starts = np.searchsorted(gsort, np.arange(NCORES * NBD))
    rank = np.empty(len(grp), np.int64)
    rank[order0] = np.arange(len(grp)) - starts[gsort]

    counts_flo = np.zeros((NCORES, NBD), np.int64)
    np.add.at(counts_flo, (core[colp < HI0], blk[colp < HI0]), 1)
    K_b = -(-counts_flo.max(axis=0) // 128)              # [NBD] lo chunks
    counts_tot = np.zeros((NCORES, NBD), np.int64)
    np.add.at(counts_tot, (core, blk), 1)
    assert (counts_tot >= 128 * K_b[None, :]).all()

    hi = (rank >= 128 * K_b[blk]).astype(np.int64)
    # hi-assigned edges must lie in the hi table's range
    assert (colp[hi == 1] >= HI0).all()
    assert (colp[hi == 0] < SPLIT).all()

    CHP = np.stack([K_b, -(-np.maximum(
        counts_tot - 128 * K_b[None, :], 0).max(axis=0) // 128)], axis=1)
    CH_TOT = int(CHP.sum())

    # span/chunk-column layout
    spans = []
    block_cols = [[] for _ in range(NBD)]
    part_col = {}                                        # (b, h) -> start col
    j = 0
    for s0 in range(0, NBD, SPAN):
        blocks = list(range(s0, min(s0 + SPAN, NBD)))
        j0 = j
        n_lo = n_hi = 0
        for h in range(2):
            for b in blocks:
                nch = int(CHP[b, h])
                part_col[(b, h)] = j
                block_cols[b].extend(range(j, j + nch))
                j += nch
                if h == 0:
                    n_lo += nch
                else:
                    n_hi += nch
        assert n_lo * 128 <= MAX_IDX_PER_GATHER
        assert n_hi * 128 <= MAX_IDX_PER_GATHER
        spans.append((j0, n_lo, n_hi, blocks))
    assert j == CH_TOT

    idx16 = np.zeros((NCORES, 128, CH_TOT * 8), np.int16)
    rl_a = np.zeros((NCORES, 128, CH_TOT), np.float32)
    nc_a = np.zeros((NCORES, 128, CH_TOT), np.float32)
    mtab = np.zeros((NCORES, 128, CH_TOT * DB), BF)

    order = np.lexsort((hi, blk, core))
    scolp, srloc, snc = colp[order], rloc[order], normcol[order]
    score, sblk, shi = core[order], blk[order], hi[order]
    bounds = np.searchsorted(
        score * (NBD * 2) + sblk * 2 + shi, np.arange(NCORES * NBD * 2 + 1))

    for c in range(NCORES):
        for b in range(NBD):
            for h in range(2):
                k = (c * NBD + b) * 2 + h
                s0, s1 = bounds[k], bounds[k + 1]
                nch = int(CHP[b, h])
                if nch == 0:
                    assert s1 == s0
                    continue
                n = nch * 128
                jc = part_col[(b, h)]
                lim = SPLIT if h == 0 else NPAD - HI0
                # spread padding indices so they don't hammer row 0
                ii = (np.arange(n, dtype=np.int64) * 97 + jc * 131) % lim
                ii[:s1 - s0] = scolp[s0:s1] - h * HI0
                rr = np.zeros(n, np.float32)
                rr[:s1 - s0] = srloc[s0:s1]
                nn = np.zeros(n, np.float32)
                nn[:s1 - s0] = snc[s0:s1]
                rl_a[c, :, jc:jc + nch] = rr.reshape(nch, 128).T
                nc_a[c, :, jc:jc + nch] = nn.reshape(nch, 128).T
                idx16[c, :, jc * 8:jc * 8 + n // 16] = _wrap_idx16(ii)
    # M tiles (layer-independent): mtab[:, j*DB:(j+1)*DB] = one-hot(rl)*norm
    dDB = np.arange(DB, dtype=np.float32)[None, None, :]
    for c in range(NCORES):
        m = (rl_a[c][:, :, None] == dDB) * nc_a[c][:, :, None]
        mtab[c] = m.astype(BF).reshape(128, CH_TOT * DB)
    return dict(spans=spans, block_cols=block_cols, CH_TOT=CH_TOT,
                idx16=idx16, rl_a=rl_a, nc_a=nc_a, mtab=mtab, ids=ids)


def _blob_offsets(CH_TOT):
    """Column layout of the packed per-core constant blob [128, NCOLS] f32."""
    off = {}
    o = 0
    for k, n in (("idx16", CH_TOT * 4), ("xT", S // 2)):
        off[k] = o
        o += n
    for l, (fin, fout) in enumerate(LAYERS):
        off[f"W{l}"] = o
        o += fout // 2
    for l, (fin, fout) in enumerate(LAYERS):
        off[f"b{l}"] = o
        o += 1
    return off, o


def _build(pre, mode="full"):
    spans = pre["spans"]
    block_cols = pre["block_cols"]
    CH_TOT = pre["CH_TOT"]
    # mode: "<base>[-repN]" where base is one of
    #   full  - real kernel
    #   nodeg - alias of full (deg phase is host-side now)
    #   nocc  - collectives -> local DMA (wrong numerics; timing ablation)
    #   noagg - skip gather+aggregation entirely (ablation)
    #   nogat - skip the dma_gather instructions only (ablation)
    #   min   - minimal output path
    # repN repeats the whole body N times for marginal-time measurement.
    reps = 1
    if "-rep" in mode:
        mode, _, r = mode.partition("-rep")
        reps = int(r)
    elif mode.startswith("rep"):
        reps, mode = int(mode[3:]), "full"
    if mode == "nodeg":
        mode = "full"
    nc = bacc.Bacc(None, num_devices=NCORES, num_swdge_queues=4)
    AGRP = [list(range(NCORES))]
    AF = mybir.ActivationFunctionType
    OP = mybir.AluOpType

    off, ncols = _blob_offsets(CH_TOT)
    blob_p = nc.declare_dram_parameter("blob", [128, ncols], F32, isOutput=False)
    out_p = nc.declare_dram_parameter("out", [LAYERS[-1][1], S], F32, isOutput=True)
    tabx_p = (nc.declare_dram_parameter("tabx", [SPLIT, WD], BF16,
                                        isOutput=False)
              if mode.startswith("gat") and mode != "gatonly" else None)
    mtab_p = nc.declare_dram_parameter("mtab", [128, CH_TOT * DB // 2], F32,
                                       isOutput=False)

    with tile.TileContext(nc, num_cores=NCORES) as tc:
        with (
            tc.tile_pool(name="const", bufs=1) as cpool,
            tc.tile_pool(name="xpool", bufs=2) as xpool,
            tc.tile_pool(name="dram", bufs=1, space="DRAM") as dpool,
            tc.tile_pool(name="psum", bufs=2, space="PSUM") as ppool,
            tc.tile_pool(name="work", bufs=3) as wpool,
        ):
            blob = cpool.tile([128, ncols], F32)
            nc.sync.dma_start(out=blob[:], in_=blob_p[:])

            idx16_sb = blob[:, off["idx16"]:off["idx16"] + CH_TOT * 4].bitcast(I16)
            xT0 = blob[:, off["xT"]:off["xT"] + S // 2].bitcast(BF16)
            W_sb = [blob[:LAYERS[l][0],
                         off[f"W{l}"]:off[f"W{l}"] + LAYERS[l][1] // 2].bitcast(BF16)
                    for l in range(4)]
            b_sb = [blob[:LAYERS[l][1], off[f"b{l}"]:off[f"b{l}"] + 1]
                    for l in range(4)]

            if mode == "min":
                o_min = xpool.tile([LAYERS[-1][1], S], F32, tag="xT")
                nc.vector.memset(o_min[:], 0.25)
                for _rep in range(reps):
                    nc.vector.tensor_tensor(
                        o_min[0:1, 0:1], blob[0:1, 0:1], blob[0:1, 0:1],
                        OP.mult)
                nc.sync.dma_start(out=out_p[:], in_=o_min[:])

            gq = [0]                 # strict round-robin SWDGE queue counter
            qload = [0, 0, 0, 0]     # greedy byte-balancing across queues

            def pick_q(nch):
                q = qload.index(min(qload))
                qload[q] += nch
                return q
            gconst = None
            if mode == "gatcomp":
                gconst = cpool.tile([128, WD], BF16)
                nc.vector.memset(gconst[:], 0.01)

            # body repetition for precise marginal-time measurement (repN)
            for _rep in range(reps if mode != "min" else 0):
                if mode in ("gatbare", "gatdve", "gatdve2", "gatact", "gatpe"):
                    o_gb = xpool.tile([LAYERS[-1][1], S], F32, tag="xT")
                    nc.vector.memset(o_gb[:], 0.25)
                    gc2 = cpool.tile([128, WD], BF16, name="gc2")
                    nc.vector.memset(gc2[:], 0.01)
                    mc2 = cpool.tile([128, 128], BF16, name="mc2")
                    nc.vector.memset(mc2[:], 0.01)
                    for l in range(4):
                        fout = LAYERS[l][1]
                        for si, (j0, n_lo, n_hi, blocks) in enumerate(spans):
                            glo = wpool.tile([128, max(n_lo, 1) * WD], BF16,
                                             tag="glo", bufs=GBUFS)
                            ghi = wpool.tile([128, max(n_hi, 1) * WD], BF16,
                                             tag="ghi", bufs=GBUFS)
                            for h, gt, nch in ((0, glo, n_lo), (1, ghi, n_hi)):
                                if nch == 0:
                                    continue
                                i16 = (j0 + (0 if h == 0 else n_lo)) * 8
                                nc.gpsimd.dma_gather(
                                    gt[:, 0:nch * WD].rearrange(
                                        "p (c w) -> p c w", w=WD),
                                    tabx_p[:], idx16_sb[:, i16:i16 + nch * 8],
                                    nch * 128, nch * 128, WD,
                                    single_packet=False, queue_num=gq[0] % 4)
                                gq[0] += 1
                            if mode == "gatact":
                                for b in blocks:
                                    for k, j in enumerate(block_cols[b]):
                                        m = wpool.tile([128, 128], BF16,
                                                       tag="m", bufs=16)
                                        nc.scalar.activation(
                                            m[:], mc2[:],
                                            mybir.ActivationFunctionType.Copy)
                            elif mode == "gatpe":
                                for b in blocks:
                                    cols = block_cols[b]
                                    ps_a = ppool.tile([fout, 128], F32,
                                                      tag="psa", bufs=4)
                                    for k, j in enumerate(cols):
                                        nc.tensor.matmul(
                                            out=ps_a[:],
                                            lhsT=gc2[:, :fout], rhs=mc2[:],
                                            start=(k == 0),
                                            stop=(k == len(cols) - 1))
                    nc.vector.tensor_tensor(
                        o_gb[0:1, 0:1], ghi[0:1, 0:1], ghi[0:1, 0:1], OP.mult)
                    nc.sync.dma_start(out=out_p[:], in_=o_gb[:])
                    continue
                xT = xT0
                # prologue: h for layer 0 from the input xT0
                h_all = wpool.tile([128, NBH * WD], BF16, tag="hall", bufs=2)
                for b in range(NBH):
                    ps_h = ppool.tile([128, LAYERS[0][1]], F32, tag="ps_small")
                    nc.tensor.matmul(
                        out=ps_h[:], lhsT=xT0[:LAYERS[0][0],
                                             b * 128:(b + 1) * 128],
                        rhs=W_sb[0], start=True, stop=True)
                    nc.scalar.activation(
                        h_all[:, b * WD:b * WD + LAYERS[0][1]], ps_h[:],
                        AF.Copy)
                for l, (fin, fout) in enumerate(LAYERS):
                    hsh = dpool.tile([S, WD], BF16, name=f"hsh{l}_{_rep}")
                    nc.sync.dma_start(
                        out=hsh.rearrange("(b p) w -> p b w", p=128),
                        in_=h_all[:].rearrange("p (b w) -> p b w", w=WD))

                    htab = dpool.tile([NPAD, WD], BF16, addr_space="Shared",
                                      name=f"htab{l}_{_rep}")
                    if mode == "nocc":
                        nc.sync.dma_start(out=htab[0:S, :], in_=hsh[:])
                    else:
                        nc.gpsimd.collective_compute(
                            "AllGather", OP.bypass, replica_groups=AGRP,
                            ins=[hsh[:]], outs=[htab[:]])

                    # aggregation: agg^T[f, d] += G[e, f]^T @ onehot[e, d]
                    xT_next = xpool.tile([fout, S], BF16 if l + 1 < len(LAYERS)
                                         else F32, tag="xT", name=f"xT{l + 1}_sb")
                    h_next = (wpool.tile([128, NBH * WD], BF16, tag="hall",
                                         bufs=2, name=f"hnext{l}_{_rep}")
                              if l + 1 < len(LAYERS) else None)
                    if mode == "noagg":
                        nc.vector.memset(xT_next[:], 0.25)
                        xT = xT_next
                        continue
                    if mode in ("gatonly", "gatpar"):
                        nc.vector.memset(xT_next[:], 0.25)
                        for si, (j0, n_lo, n_hi, blocks) in enumerate(spans):
                            nch_s = n_lo + n_hi
                            g = wpool.tile([128, nch_s * WD], BF16, tag="g",
                                           bufs=GBUFS)
                            tlo = htab[0:SPLIT, :] if mode == "gatonly" \
                                else tabx_p[:]
                            thi = htab[SPLIT:NPAD, :] if mode == "gatonly" \
                                else tabx_p[:]
                            for h, tab, o0, nch in (
                                    (0, tlo, 0, n_lo),
                                    (1, thi, n_lo * WD, n_hi)):
                                if nch == 0:
                                    continue
                                i16 = (j0 + (0 if h == 0 else n_lo)) * 8
                                nc.gpsimd.dma_gather(
                                    g[:, o0:o0 + nch * WD].rearrange(
                                        "p (c w) -> p c w", w=WD),
                                    tab, idx16_sb[:, i16:i16 + nch * 8],
                                    nch * 128, nch * 128, WD,
                                    single_packet=False, queue_num=gq[0] % 4)
                                gq[0] += 1
                        xT = xT_next
                        continue
                    for si, (j0, n_lo, n_hi, blocks) in enumerate(spans):
                        glo = wpool.tile([128, max(n_lo, 1) * WD], BF16,
                                         tag="glo", bufs=GBUFS + 2)
                        ghi = wpool.tile([128, max(n_hi, 1) * WD], BF16,
                                         tag="ghi", bufs=GBUFS)
                        for h, tab, gt, nch in ((0, htab[0:SPLIT, :], glo, n_lo),
                                                (1, htab[HI0:NPAD, :],
                                                 ghi, n_hi)):
                            if nch == 0:
                                continue
                            if mode == "nogat":
                                nc.vector.memset(gt[:, 0:1], 0.25)
                                continue
                            if mode == "gatcomp":
                                tab = tabx_p[:]
                            i16 = (j0 + (0 if h == 0 else n_lo)) * 8
                            nc.gpsimd.dma_gather(
                                gt[:, 0:nch * WD].rearrange(
                                    "p (c w) -> p c w", w=WD),
                                tab, idx16_sb[:, i16:i16 + nch * 8],
                                nch * 128, nch * 128, WD,
                                single_packet=False,
                                queue_num=0 if os.environ.get("GQ1")
                                else pick_q(nch))
                            gq[0] += 1
                        nch_s = n_lo + n_hi
                        m_span = wpool.tile([128, nch_s * DB], BF16,
                                            tag="m", bufs=3)
                        nc.sync.dma_start(
                            out=m_span[:],
                            in_=mtab_p[:, j0 * DB // 2:(j0 + nch_s) * DB // 2]
                            .bitcast(BF16))
                        for b in blocks:
                            cols = block_cols[b]
                            ps_a = ppool.tile([fout, DB], F32, tag="psa", bufs=4)
                            for k, j in enumerate(cols):
                                if mode == "gatcomp":
                                    gsrc, o0 = gconst, 0
                                elif j - j0 < n_lo:
                                    gsrc, o0 = glo, (j - j0) * WD
                                else:
                                    gsrc, o0 = ghi, (j - j0 - n_lo) * WD
                                nc.tensor.matmul(
                                    out=ps_a[:], lhsT=gsrc[:, o0:o0 + fout],
                                    rhs=m_span[:, (j - j0) * DB:
                                               (j - j0 + 1) * DB],
                                    start=(k == 0),
                                    stop=(k == len(cols) - 1))
                            # epilogue: +bias, ReLU (norm fully in M)
                            nc.scalar.activation(
                                xT_next[:, b * DB:(b + 1) * DB], ps_a[:],
                                AF.Relu if l + 1 < len(LAYERS) else AF.Identity,
                                bias=b_sb[l][:, 0:1])
                            if l + 1 < len(LAYERS):
                                # next layer's h for this block rides the agg
                                # phase (PE/Act have slack; hides the h phase
                                # and starts the collective earlier)
                                fin2, fout2 = LAYERS[l + 1]
                                ps_h = ppool.tile([128, fout2], F32,
                                                  tag="ps_small")
                                nc.tensor.matmul(
                                    out=ps_h[:],
                                    lhsT=xT_next[:fin2,
                                                 b * 128:(b + 1) * 128],
                                    rhs=W_sb[l + 1], start=True, stop=True)
                                nc.scalar.activation(
                                    h_next[:, b * WD:b * WD + fout2],
                                    ps_h[:], AF.Copy)
                    xT = xT_next
                    h_all = h_next

                nc.sync.dma_start(out=out_p[:], in_=xT[:])
    nc.finalize()   # Bacc: reg alloc + event-sem wait splitting
    return nc


def _make_in_maps(x, pre, W0, b0, W1, b1, W2, b2, W3, b3):
    CH_TOT, idx16, mtab, ids = (pre["CH_TOT"], pre["idx16"], pre["mtab"],
                                pre["ids"])
    rl_a, nc_a = pre["rl_a"], pre["nc_a"]
    off, ncols = _blob_offsets(CH_TOT)
    Ws = [np.asarray(W, np.float32).astype(BF) for W in (W0, W1, W2, W3)]
    bs = [np.asarray(b, np.float32).reshape(-1) for b in (b0, b1, b2, b3)]

    in_maps = []
    for c in range(NCORES):
        blob = np.zeros((128, ncols), np.float32)
        blob[:, off["idx16"]:off["idx16"] + CH_TOT * 4] = idx16[c].view(np.float32)
        xb = np.zeros((128, S), BF)
        valid = ids[c] >= 0
        xb[:, valid] = x[ids[c][valid]].T.astype(BF)
        blob[:, off["xT"]:off["xT"] + S // 2] = xb.view(np.float32)
        for l, (fin, fout) in enumerate(LAYERS):
            blob[:fin, off[f"W{l}"]:off[f"W{l}"] + fout // 2] = Ws[l].view(np.float32)
            blob[:fout, off[f"b{l}"]] = bs[l]
        in_maps.append({"blob": blob,
                        "mtab": mtab[c].view(np.float32)})
    return in_maps


def kernel(x, edge_index, edge_weight, W0, b0, W1, b1, W2, b2, W3, b3):
    global LAST_RESULTS
    x = np.ascontiguousarray(np.asarray(x, np.float32))
    pre = _preprocess(np.asarray(edge_index), np.asarray(edge_weight))

    nc = _build(pre)

    in_maps = _make_in_maps(x, pre, W0, b0, W1, b1, W2, b2, W3, b3)

    res = run_bass_kernel_spmd(nc, in_maps, core_ids=list(range(NCORES)),
                               trace=TRACE, trace_kwargs=dict(TRACE_KW))
    LAST_RESULTS = res
    ids = pre["ids"]
    out = np.zeros((N_NODES, LAYERS[-1][1]), np.float32)
    for c in range(NCORES):
        valid = ids[c] >= 0
        out[ids[c][valid]] = res.results[c]["out"][:, valid].T
    return np.ascontiguousarray(out)
